# revision 1
# baseline (speedup 1.0000x reference)
"""NeuralSort relaxed-permutation kernel for 8 Trainium2 NeuronCores.

out[b, i, j] = softmax_i( s_i * scaling_j - B_i ),  s = -scores[b]
  scaling_j = n - 1 - 2j   =>  z[i,j] = c_j * x_i - B_i  with x = scores[b],
  c_j = -(n - 1 - 2j) = 2j + 1 - n
  B_i = sum_k |x_i - x_k| = x_i*(n - 2*cnt_i) - S + 2*t_i
        cnt_i = #{k: x_k > x_i},  t_i = sum_{k: x_k > x_i} x_k,  S = sum_k x_k

Sharding: core c -> (batch b = c//2, j-half h = c%2). Each core emits the
full-i (n) by half-j (n/2) slab of batch b.

Per-core pipeline (all matmuls bf16 @ 1 cyc/row, exact via hi/mid/lo splits):
  P: comparison tiles split across DVE (is_lt -> {0,1}) and ACT (Sign ->
     {-1,0,1}); PE reduces them with [ones|x_hi|x_lo] stationaries into
     cnt/t rows; combine to the B half in column layout; AllGather [B; r1b]
     within the batch pair (r1b = #below - #above, a signed rank).
  M-bound: i's are bucketed into 128 rank ranges by r1b (mask tiles + PE
     mask-matmul -> per-bucket mean (x_bar, B_bar)). z evaluated at the 128
     bucket means underestimates each column max by <~40 (z is flat near
     its optimum in rank space; B_bar >= f(x_bar) by convexity of
     f(x) = sum_k|x - x_k|), which is all the exp shift needs.
  S: per 128-j chunk: M'_j = rowmax of l9-slice^T @ rep9 (one tiny matmul +
     one [128,128] DVE reduce); z = c_j x_i - B_i via K=9 bf16 stacked
     matmul into PSUM (j on partitions, i on free); ACT exp(z - M') with
     accum_out -> D_j; Q_j = M'_j + ln(D_j) (Ln batched per j group).
  O: z' = c_j x_i - Q_j - B_i via K=12 bf16 matmul (i on partitions, j on
     free; B folded in so exp needs no bias and spans ic-pairs); ACT exp
     from PSUM -> final values in SBUF; one 4 MiB DMA per 16 i-chunks,
     staged per 512-j group so DMA overlaps the next group's stats.
"""

from contextlib import ExitStack

import numpy as np
import ml_dtypes

import concourse.bass as bass
import concourse.tile as tile
from concourse import bacc, mybir
from concourse.bass_utils import run_bass_kernel_spmd

F32 = mybir.dt.float32
BF16 = mybir.dt.bfloat16
AF = mybir.ActivationFunctionType
ALU = mybir.AluOpType

N_CORES = 8
P = 128


def _bf(x):
    return np.asarray(x, dtype=ml_dtypes.bfloat16)


def _split3(x):
    x = np.asarray(x, dtype=np.float32)
    h = _bf(x)
    r = x - h.astype(np.float32)
    m = _bf(r)
    l = _bf(r - m.astype(np.float32))
    return h, m, l


def _split2(x):
    x = np.asarray(x, dtype=np.float32)
    h = _bf(x)
    l = _bf(x - h.astype(np.float32))
    return h, l


# K-row pairing for the z matmuls (z = sum_k lhsT_row_k * rhs_row_k).
# Stacks put PE-transposed rows first (matmul outputs must land at partition
# base 0); DMA-filled rows follow (DMA can target any partition).
# Stats: lhsT rows from [chi, clo, ones]; rhs rows from [Bh,Bm,Bl,xh,xm,xl].
SEL_L_S = np.zeros((3, 9), np.float32)
SEL_R_S = np.zeros((6, 9), np.float32)
_PAIRS_S = [
    (0, 3, 1.0),   # c_hi * x_h
    (2, 0, -1.0),  # 1 * -B_h
    (1, 3, 1.0),   # c_lo * x_h
    (0, 4, 1.0),   # c_hi * x_m
    (2, 1, -1.0),  # 1 * -B_m
    (1, 4, 1.0),   # c_lo * x_m
    (0, 5, 1.0),   # c_hi * x_l
    (2, 2, -1.0),  # 1 * -B_l
    (1, 5, 1.0),   # c_lo * x_l
]
for k, (ls, rs, w) in enumerate(_PAIRS_S):
    SEL_L_S[ls, k] = 1.0
    SEL_R_S[rs, k] = w

# Output: z'' = c_j x_i - Q_j - B_i, K=12. lhsT rows from
# [xh, xm, xl, ones, Bh, Bm, Bl]; rhs rows from [Qh, Qm, Ql, chi, clo, ones].
# Folding B into the matmul removes the ACT bias, letting exp span ic-pairs.
SEL_L_O = np.zeros((7, 12), np.float32)
SEL_R_O = np.zeros((6, 12), np.float32)
_PAIRS_O = [
    (0, 3, 1.0),   # x_h * c_hi
    (3, 0, -1.0),  # 1 * -Q_h
    (4, 5, -1.0),  # B_h * -1
    (0, 4, 1.0),   # x_h * c_lo
    (1, 3, 1.0),   # x_m * c_hi
    (3, 1, -1.0),  # 1 * -Q_m
    (5, 5, -1.0),  # B_m * -1
    (1, 4, 1.0),   # x_m * c_lo
    (2, 3, 1.0),   # x_l * c_hi
    (3, 2, -1.0),  # 1 * -Q_l
    (6, 5, -1.0),  # B_l * -1
    (2, 4, 1.0),   # x_l * c_lo
]
for k, (ls, rs, w) in enumerate(_PAIRS_O):
    SEL_L_O[ls, k] = 1.0
    SEL_R_O[rs, k] = w


def _dve_ks(nkc):
    """Comparison chunks assigned to DVE (rest go to ACT as Sign)."""
    if nkc >= 8:
        return [k for k in range(nkc) if k % 8 < 5]
    return [k for k in range(nkc) if k % 2 == 0]


def build_nc(n=4096, mode="pair", num_devices=N_CORES):
    """mode: "pair" (8-core, AllGather B halves); "single" (1-core debug:
    full j/i ranges, no collective); "timing" (pair shapes, collective
    replaced by local row copies -- for the single-core timeline model)."""
    single = mode == "single"
    use_collective = mode == "pair"
    nj = n if single else n // 2    # output columns per core
    nih = n if single else n // 2   # i-range whose B this core computes
    nkc = n // P                    # k-chunks in the G pass
    njc = nj // P                   # 128-wide j-chunks for stats
    nic = n // P                    # 128-wide i-chunks for output
    jg = 512                        # output j staging group width
    n_jg = nj // jg
    jcs_per_g = jg // P
    icg = min(8, nic)               # i-chunks per output DMA
    niq = max(2, n // 1024)         # stats i-quarter count
    iq = n // niq                   # stats free-dim per z tile (<=1024)

    nc = bacc.Bacc(
        "TRN2", target_bir_lowering=False, debug=False, num_devices=num_devices
    )

    def din(name, shape, dt=F32):
        return nc.dram_tensor(name, shape, dt, kind="ExternalInput").ap()

    n_dve = len(_dve_ks(nkc))  # comparison chunks on DVE; rest ACT (Sign)

    xs4 = din("xs4", [4, n], BF16)        # rows [xh; xm; xl; ones]
    xsh3 = din("xsh3", [3, nih], BF16)    # x splits of this core's i-half
    xso3 = din("xso3", [3, nih], BF16)    # x splits of the partner's i-half
    cs3 = din("cs3", [3, nj], BF16)       # rows [chi; clo; ones]
    xcol = din("xcol", [P, nkc], F32)
    xhc = din("xhc", [P, nih // P], F32)  # x of this core's i-half, col layout
    blhs = din("blhs", [P, 3 * nkc], BF16)
    xc2d = din("xc2d", [P, 2 * n_dve], BF16)  # [xch | xcl], DVE-chunk cols
    i128 = din("i128", [P, P], BF16)
    lob = din("lob", [P, P], F32)    # rank-bucket lower bounds along free
    hib = din("hib", [P, P], F32)    # rank-bucket upper bounds along free
    xballh = din("xballh", [P, nih // P, 5], BF16)  # own chunks [xch,xcl,0,0,1]
    sel_l_s = din("sel_l_s", [3, 9], BF16)
    sel_r_s = din("sel_r_s", [6, 9], BF16)
    sel_l_o = din("sel_l_o", [7, 12], BF16)
    sel_r_o = din("sel_r_o", [6, 12], BF16)
    ones3 = din("ones3", [3, P], BF16)
    onesc = din("onesc", [P, 1], BF16)

    out = nc.dram_tensor("out", [n, nj], F32, kind="ExternalOutput").ap()

    # exchange payload: [B-half || 128x5 rank-bucket partial sums]
    npay = nih + 5 * P
    bh_dram = nc.dram_tensor("bh_dram", [1, npay], F32).ap()
    nhalves = 1 if single else 2
    bfull_dram = nc.dram_tensor("bfull_dram", [nhalves, npay], F32).ap()
    b3_dram = nc.dram_tensor("b3_dram", [6, nih], F32).ap()
    bspl_dram = nc.dram_tensor("bspl_dram", [3, n], BF16).ap()
    bsplh_dram = nc.dram_tensor("bsplh_dram", [3, nih], BF16).ap()
    bsplo_dram = nc.dram_tensor("bsplo_dram", [3, nih], BF16).ap()
    groups = [[2 * p, 2 * p + 1] for p in range(max(1, num_devices // 2))]

    def mm512(out_ap, lhsT, rhs, start=True, stop=True):
        """matmul with the moving dim split into <=512-column chunks."""
        nfree = rhs.shape[-1]
        assert out_ap.shape[-1] == nfree
        for o in range(0, nfree, 512):
            e = min(o + 512, nfree)
            nc.tensor.matmul(
                out_ap[..., o:e], lhsT, rhs[..., o:e], start=start, stop=stop
            )

    with tile.TileContext(nc) as tc, ExitStack() as ctx:
        cpool = ctx.enter_context(tc.tile_pool(name="consts", bufs=1))

        def load(pool, ap_dram, shape, dt, name):
            t = pool.tile(shape, dt, tag=name)
            nc.sync.dma_start(out=t[:], in_=ap_dram)
            return t

        # long-lived constants
        i128_s = load(cpool, i128, [P, P], BF16, "i128")
        sel_r_o_s = load(cpool, sel_r_o, [6, 12], BF16, "sel_r_o")
        bcol = cpool.tile([P, nic], F32, tag="bcol")
        l9 = cpool.tile([9, nj], BF16, tag="l9")
        r9 = [
            cpool.tile([9, nih], BF16, tag=f"r9_{h}", name=f"r9_{h}")
            for h in range(nhalves)
        ]
        l9o = cpool.tile([12, n], BF16, tag="l9o")
        r9o = cpool.tile([12, nj], BF16, tag="r9o")
        src_o = cpool.tile([6, nj], BF16, tag="src_o")
        qcol = cpool.tile([P, njc], F32, tag="qcol")
        rep9 = cpool.tile([9, P], BF16, tag="rep9")

        with tc.tile_pool(name="prep", bufs=1) as pp_s:
            xcol_s = load(pp_s, xcol, [P, nkc], F32, "xcol")
            xhc_s = load(pp_s, xhc, [P, nih // P], F32, "xhc")
            blhs_s = load(pp_s, blhs, [P, 3 * nkc], BF16, "blhs")
            xc2d_s = load(pp_s, xc2d, [P, 2 * n_dve], BF16, "xc2d")
            sel_l_s_s = load(pp_s, sel_l_s, [3, 9], BF16, "sel_l_s")
            sel_r_s_s = load(pp_s, sel_r_s, [6, 9], BF16, "sel_r_s")
            sel_l_o_s = load(pp_s, sel_l_o, [7, 12], BF16, "sel_l_o")
            ones3_s = load(pp_s, ones3, [3, P], BF16, "ones3")
            onesc_s = load(pp_s, onesc, [P, 1], BF16, "onesc")
            xsh = load(pp_s, xsh3, [3, nih], BF16, "xsh")

            # ---- Phase P: B over this core's i-half ----
            xb = pp_s.tile([P, nih], F32, tag="xb")
            with tc.tile_pool(name="pp", bufs=1, space="PSUM") as pp:
                xbp = pp.tile([P, nih], F32)
                mm512(xbp[:], ones3_s[:], xsh[:])
                nc.vector.tensor_copy(out=xb[:], in_=xbp[:])

            # comparison pass interleaved between DVE (is_lt -> G in {0,1})
            # and ACT (Sign -> sgn in {-1,0,1}); interleaving keeps both
            # engines fed since PE drains g tiles in program order. For the
            # DVE set, sum_k |x_i-x_k| = x_i*(nD - 2*cntD) - SD + 2*tD; for
            # the ACT set it's x_i*sgnS - tS. Tie terms vanish either way.
            dve_ks = set(_dve_ks(nkc))
            nxcol = pp_s.tile([P, nkc], F32, tag="nxcol")
            nc.vector.tensor_scalar_mul(nxcol[:], xcol_s[:], -1.0)
            b3 = pp_s.tile([3, nih], F32, tag="b3")
            b3s = pp_s.tile([3, nih], F32, tag="b3s")
            with (
                tc.tile_pool(name="bp", bufs=1, space="PSUM") as bp,
                tc.tile_pool(name="gp", bufs=4) as gp,
            ):
                bpsum = bp.tile([3, nih], F32)
                bpsum2 = bp.tile([3, nih], F32)
                ndve_seen = nact_seen = 0
                for k in range(nkc):
                    g = gp.tile([P, nih], BF16, tag="g")
                    if k in dve_ks:
                        ndve_seen += 1
                        nc.vector.tensor_scalar(
                            out=g[:],
                            in0=xb[:],
                            scalar1=xcol_s[:, k : k + 1],
                            scalar2=None,
                            op0=ALU.is_lt,
                        )
                        mm512(
                            bpsum[:],
                            blhs_s[:, 3 * k : 3 * k + 3],
                            g[:],
                            start=(ndve_seen == 1),
                            stop=(ndve_seen == n_dve),
                        )
                    else:
                        nact_seen += 1
                        nc.scalar.activation(
                            out=g[:],
                            in_=xb[:],
                            func=AF.Sign,
                            bias=nxcol[0:P, k : k + 1],
                        )
                        mm512(
                            bpsum2[:],
                            blhs_s[:, 3 * k : 3 * k + 3],
                            g[:],
                            start=(nact_seen == 1),
                            stop=(nact_seen == nkc - n_dve),
                        )
                nc.vector.tensor_copy(out=b3[:], in_=bpsum[:])
                nc.vector.tensor_copy(out=b3s[:], in_=bpsum2[:])

            # roundtrip the six rows through DRAM into column layout (one
            # readback DMA for all six)
            nihc = nih // P
            nc.sync.dma_start(out=b3_dram[0:3, :], in_=b3[:])
            nc.sync.dma_start(out=b3_dram[3:6, :], in_=b3s[:])
            bc_all = pp_s.tile([P, 6 * nihc], F32, tag="bc_all")
            nc.sync.dma_start(
                out=bc_all[:],
                in_=b3_dram.rearrange("r (t p) -> p (r t)", p=P),
            )
            cntc = bc_all[:, 0 * nihc : 1 * nihc]
            thc = bc_all[:, 1 * nihc : 2 * nihc]
            tlc = bc_all[:, 2 * nihc : 3 * nihc]
            sgnc = bc_all[:, 3 * nihc : 4 * nihc]
            tshc = bc_all[:, 4 * nihc : 5 * nihc]
            tslc = bc_all[:, 5 * nihc : 6 * nihc]

            # -SD (sum of x over the DVE-chunk k's)
            sneg = pp_s.tile([1, 1], F32, tag="sneg")
            with tc.tile_pool(name="sp", bufs=1, space="PSUM") as sp:
                sxp = sp.tile([1, 2 * n_dve], F32)
                nc.tensor.matmul(sxp[:], onesc_s[:], xc2d_s[:], start=True, stop=True)
                ssum = pp_s.tile([1, 1], F32, tag="ssum")
                nc.vector.tensor_reduce(
                    out=ssum[:], in_=sxp[:], axis=mybir.AxisListType.X, op=ALU.add
                )
                nc.vector.tensor_scalar_mul(sneg[:], ssum[:], -1.0)
            snegc = pp_s.tile([P, 1], F32, tag="snegc")
            nc.gpsimd.partition_broadcast(snegc[:], sneg[0:1, 0:1])

            # B = x*(nD - 2*cntD + sgnS) - SD + 2*(tDh+tDl) - (tSh+tSl)
            nD = float(n_dve * P)
            r1 = pp_s.tile([P, nihc], F32, tag="r1")
            nc.vector.tensor_scalar(
                out=r1[:],
                in0=cntc[:],
                scalar1=-2.0,
                scalar2=nD,
                op0=ALU.mult,
                op1=ALU.add,
            )
            r1b = pp_s.tile([P, nihc], F32, tag="r1b")
            nc.vector.tensor_tensor(out=r1b[:], in0=r1[:], in1=sgnc[:], op=ALU.add)
            r2 = pp_s.tile([P, nihc], F32, tag="r2")
            nc.vector.tensor_tensor(out=r2[:], in0=xhc_s[:], in1=r1b[:], op=ALU.mult)
            tt = pp_s.tile([P, nihc], F32, tag="tt")
            nc.vector.tensor_tensor(out=tt[:], in0=thc[:], in1=tlc[:], op=ALU.add)
            u1 = pp_s.tile([P, nihc], F32, tag="u1")
            nc.vector.scalar_tensor_tensor(
                out=u1[:], in0=tt[:], scalar=2.0, in1=r2[:], op0=ALU.mult, op1=ALU.add
            )
            tts = pp_s.tile([P, nihc], F32, tag="tts")
            nc.vector.tensor_tensor(out=tts[:], in0=tshc[:], in1=tslc[:], op=ALU.add)
            u2 = pp_s.tile([P, nihc], F32, tag="u2")
            nc.vector.tensor_tensor(out=u2[:], in0=u1[:], in1=tts[:], op=ALU.subtract)
            bhalfc = pp_s.tile([P, nihc], F32, tag="bhalfc")
            nc.vector.tensor_scalar(
                out=bhalfc[:],
                in0=u2[:],
                scalar1=snegc[:, 0:1],
                scalar2=None,
                op0=ALU.add,
            )

            # ---- own-half B splits: available without the exchange, so the
            # own-half stats start while the AllGather is in flight ----
            def col_splits(src_col, dst_tile, w, tg):
                s0 = dst_tile[:, 0 * w : 1 * w]
                s1 = dst_tile[:, 1 * w : 2 * w]
                s2 = dst_tile[:, 2 * w : 3 * w]
                t1 = pp_s.tile([P, w], F32, tag=f"{tg}_t1", name=f"{tg}_t1")
                t2 = pp_s.tile([P, w], F32, tag=f"{tg}_t2", name=f"{tg}_t2")
                nc.vector.tensor_copy(out=s0, in_=src_col)
                nc.vector.tensor_tensor(
                    out=t1[:], in0=src_col, in1=s0, op=ALU.subtract
                )
                nc.vector.tensor_copy(out=s1, in_=t1[:])
                nc.vector.tensor_tensor(
                    out=t2[:], in0=t1[:], in1=s1, op=ALU.subtract
                )
                nc.vector.tensor_copy(out=s2, in_=t2[:])

            bsh_all = pp_s.tile([P, 3 * nihc], BF16, tag="bsh_all")
            col_splits(bhalfc[:], bsh_all, nihc, "so")
            nc.sync.dma_start(
                out=bsplh_dram.rearrange("s (t p) -> p (s t)", p=P), in_=bsh_all[:]
            )

            # ---- own-half rank-bucket partial sums (pre-exchange) ----
            lob_s = load(pp_s, lob, [P, P], F32, "lob")
            hib_s = load(pp_s, hib, [P, P], F32, "hib")
            xballh_s = load(pp_s, xballh, [P, nihc, 5], BF16, "xballh")
            nc.vector.tensor_copy(out=xballh_s[:, :, 2], in_=bsh_all[:, 0:nihc])
            nc.vector.tensor_copy(
                out=xballh_s[:, :, 3], in_=bsh_all[:, nihc : 2 * nihc]
            )
            reps_own = pp_s.tile([P, 5], F32, tag="reps_own")
            with (
                tc.tile_pool(name="repp0", bufs=1, space="PSUM") as repp0,
                tc.tile_pool(name="mkp0", bufs=4) as mkp0,
            ):
                repso_p = repp0.tile([P, 5], F32)
                for ch in range(nihc):
                    m1 = mkp0.tile([P, P], BF16, tag="m1")
                    nc.vector.tensor_scalar(
                        out=m1[:],
                        in0=lob_s[:],
                        scalar1=r1b[:, ch : ch + 1],
                        scalar2=None,
                        op0=ALU.is_le,
                    )
                    msk = mkp0.tile([P, P], BF16, tag="msk")
                    nc.vector.scalar_tensor_tensor(
                        out=msk[:],
                        in0=hib_s[:],
                        scalar=r1b[:, ch : ch + 1],
                        in1=m1[:],
                        op0=ALU.is_gt,
                        op1=ALU.mult,
                    )
                    nc.tensor.matmul(
                        repso_p[:],
                        msk[:],
                        xballh_s[:, ch, :],
                        start=(ch == 0),
                        stop=(ch == nihc - 1),
                    )
                nc.vector.tensor_copy(out=reps_own[:], in_=repso_p[:])

            # ---- exchange [B-half || rep partials] within the batch pair ----
            nc.sync.dma_start(
                out=bh_dram[0:1, 0:nih].rearrange("a (t p) -> p (a t)", p=P),
                in_=bhalfc[:],
            )
            nc.sync.dma_start(
                out=bh_dram[0:1, nih:npay].rearrange("a (p f) -> p (a f)", p=P),
                in_=reps_own[:],
            )
            if use_collective:
                nc.gpsimd.collective_compute(
                    "AllGather",
                    ALU.bypass,
                    replica_groups=groups,
                    ins=[bh_dram],
                    outs=[bfull_dram],
                )
            else:
                for hh in range(nhalves):
                    nc.sync.dma_start(out=bfull_dram[hh : hh + 1, :], in_=bh_dram)

            # column-layout readback of the full B (true i-order) and the
            # combined bucket sums (sum over both halves = full buckets)
            reps_full = pp_s.tile([P, 5], F32, tag="reps_full")
            for hh in range(nhalves):
                hsl = slice(hh * nihc, (hh + 1) * nihc)
                nc.sync.dma_start(
                    out=bcol[:, hsl],
                    in_=bfull_dram[hh, 0:nih].rearrange("(t p) -> p t", p=P),
                )
                rpart = pp_s.tile([P, 5], F32, tag=f"rpart{hh}", name=f"rpart{hh}")
                nc.sync.dma_start(
                    out=rpart[:],
                    in_=bfull_dram[hh, nih:npay].rearrange("(p f) -> p f", p=P),
                )
                if hh == 0:
                    nc.vector.tensor_copy(out=reps_full[:], in_=rpart[:])
                else:
                    nc.vector.tensor_tensor(
                        out=reps_full[:], in0=reps_full[:], in1=rpart[:],
                        op=ALU.add,
                    )

            # other-half B, position-free: other = (half0 + half1) - own
            if nhalves == 2:
                otherc = pp_s.tile([P, nihc], F32, tag="otherc")
                nc.vector.tensor_tensor(
                    out=otherc[:],
                    in0=bcol[:, 0:nihc],
                    in1=bcol[:, nihc : 2 * nihc],
                    op=ALU.add,
                )
                nc.vector.tensor_tensor(
                    out=otherc[:], in0=otherc[:], in1=bhalfc[:], op=ALU.subtract
                )
                bso_all = pp_s.tile([P, 3 * nihc], BF16, tag="bso_all")
                col_splits(otherc[:], bso_all, nihc, "oo")
                nc.sync.dma_start(
                    out=bsplo_dram.rearrange("s (t p) -> p (s t)", p=P),
                    in_=bso_all[:],
                )
            bs_all = pp_s.tile([P, 3 * nic], BF16, tag="bs_all")
            col_splits(bcol[:], bs_all, nic, "sf")
            nc.sync.dma_start(
                out=bspl_dram.rearrange("s (t p) -> p (s t)", p=P), in_=bs_all[:]
            )

            # ---- rank-bucket representatives -> rep9 stack ----
            # reps_full rows: [sum xh, sum xl, sum Bh, sum Bm, count]
            reps = reps_full
            cnt1 = pp_s.tile([P, 1], F32, tag="cnt1")
            nc.vector.tensor_scalar_max(cnt1[:], reps[:, 4:5], 1.0)
            rc = pp_s.tile([P, 1], F32, tag="rc")
            nc.vector.reciprocal(rc[:], cnt1[:])
            repx = pp_s.tile([P, 1], F32, tag="repx")
            nc.vector.tensor_tensor(
                out=repx[:], in0=reps[:, 0:1], in1=reps[:, 1:2], op=ALU.add
            )
            nc.vector.tensor_tensor(
                out=repx[:], in0=repx[:], in1=rc[:], op=ALU.mult
            )
            repb = pp_s.tile([P, 1], F32, tag="repb")
            nc.vector.tensor_tensor(
                out=repb[:], in0=reps[:, 2:3], in1=reps[:, 3:4], op=ALU.add
            )
            nc.vector.tensor_tensor(
                out=repb[:], in0=repb[:], in1=rc[:], op=ALU.mult
            )
            # empty bucket -> push its line to -inf via a huge B
            iz = pp_s.tile([P, 1], F32, tag="iz")
            nc.vector.tensor_scalar(
                out=iz[:], in0=reps[:, 4:5], scalar1=0.5, scalar2=None,
                op0=ALU.is_le,
            )
            nc.vector.scalar_tensor_tensor(
                out=repb[:], in0=iz[:], scalar=1e30, in1=repb[:],
                op0=ALU.mult, op1=ALU.add,
            )
            # split cols [Bh2 Bm2 Bl0 xh2 xl2 x0] matching sel_r_s order
            rs6 = pp_s.tile([P, 6], BF16, tag="rs6")
            rtmp = pp_s.tile([P, 1], F32, tag="rep_rt")
            nc.vector.tensor_copy(out=rs6[:, 0:1], in_=repb[:])
            nc.vector.tensor_tensor(
                out=rtmp[:], in0=repb[:], in1=rs6[:, 0:1], op=ALU.subtract
            )
            nc.vector.tensor_copy(out=rs6[:, 1:2], in_=rtmp[:])
            nc.vector.memset(rs6[:, 2:3], 0.0)
            nc.vector.tensor_copy(out=rs6[:, 3:4], in_=repx[:])
            nc.vector.tensor_tensor(
                out=rtmp[:], in0=repx[:], in1=rs6[:, 3:4], op=ALU.subtract
            )
            nc.vector.tensor_copy(out=rs6[:, 4:5], in_=rtmp[:])
            nc.vector.memset(rs6[:, 5:6], 0.0)
            with tc.tile_pool(name="repp", bufs=1, space="PSUM") as repp:
                p6 = repp.tile([6, P], F32)
                nc.tensor.matmul(p6[:], rs6[:], i128_s[:], start=True, stop=True)
                srep = pp_s.tile([6, P], BF16, tag="srep")
                nc.vector.tensor_copy(out=srep[:], in_=p6[:])
                p9r = repp.tile([9, P], F32)
                nc.tensor.matmul(
                    p9r[:], sel_r_s_s[:], srep[:], start=True, stop=True
                )
                nc.vector.tensor_copy(out=rep9[:], in_=p9r[:])

            # stats rhs source stacks: [0]=own half, [1]=partner half
            # (max/sum over i are order-invariant, so halves need not be in
            # true i-order)
            src_s = [
                pp_s.tile([6, nih], BF16, tag=f"src_s{h}", name=f"src_s{h}")
                for h in range(nhalves)
            ]
            nc.sync.dma_start(out=src_s[0][0:3, :], in_=bsplh_dram)
            nc.sync.dma_start(out=src_s[0][3:6, :], in_=xsh3)
            if nhalves == 2:
                nc.sync.dma_start(out=src_s[1][0:3, :], in_=bsplo_dram)
                nc.sync.dma_start(out=src_s[1][3:6, :], in_=xso3)

            # stats lhsT stack [9, nj] and rhs stacks [9, nih] per half
            src_l = pp_s.tile([3, nj], BF16, tag="src_l")
            nc.sync.dma_start(out=src_l[:], in_=cs3)
            with tc.tile_pool(name="stk", bufs=1, space="PSUM") as stk:
                p9 = stk.tile([9, max(nj, nih)], F32)
                mm512(p9[:, 0:nj], sel_l_s_s[:], src_l[:])
                nc.vector.tensor_copy(out=l9[:], in_=p9[:, 0:nj])
                for h in range(nhalves):
                    mm512(p9[:, 0:nih], sel_r_s_s[:], src_s[h][:])
                    nc.vector.tensor_copy(out=r9[h][:], in_=p9[:, 0:nih])

            # output lhsT stack [9, n] from [xh; xm; xl; ones]
            src_lo = pp_s.tile([7, n], BF16, tag="src_lo")
            nc.sync.dma_start(out=src_lo[0:4, :], in_=xs4)
            nc.sync.dma_start(out=src_lo[4:7, :], in_=bspl_dram)
            with tc.tile_pool(name="stko", bufs=1, space="PSUM") as stko:
                for hh in range(2):
                    p9o = stko.tile([12, n // 2], F32)
                    mm512(
                        p9o[:],
                        sel_l_o_s[:],
                        src_lo[:, hh * (n // 2) : (hh + 1) * (n // 2)],
                    )
                    nc.vector.tensor_copy(
                        out=l9o[:, hh * (n // 2) : (hh + 1) * (n // 2)], in_=p9o[:]
                    )

        # ---------------- Phases S+O interleaved over j groups ----------------
        nc.sync.dma_start(out=src_o[3:6, :], in_=cs3)

        spool = ctx.enter_context(tc.tile_pool(name="sz", bufs=2, space="PSUM"))
        opool = ctx.enter_context(tc.tile_pool(name="oz", bufs=2, space="PSUM"))
        epool = ctx.enter_context(tc.tile_pool(name="escr", bufs=3))
        mpool = ctx.enter_context(tc.tile_pool(name="m", bufs=8))
        qspl = ctx.enter_context(tc.tile_pool(name="qspl", bufs=2))
        outp = ctx.enter_context(tc.tile_pool(name="outp", bufs=3))

        for g in range(n_jg):
            gs = g * jcs_per_g
            mg = mpool.tile([P, jcs_per_g], F32, tag="mg", name="mg")
            dg = mpool.tile([P, jcs_per_g], F32, tag="dg", name="dg")
            for jci in range(jcs_per_g):
                jc = gs + jci
                lhs = l9[:, jc * P : (jc + 1) * P]
                zrp = spool.tile([P, P], F32, tag="sz")
                nc.tensor.matmul(zrp[:], lhs, rep9[:], start=True, stop=True)
                m = mpool.tile([P, 1], F32, tag="m")
                nc.vector.tensor_reduce(
                    out=m[:], in_=zrp[:], axis=mybir.AxisListType.X, op=ALU.max
                )
                nc.vector.tensor_copy(out=mg[:, jci : jci + 1], in_=m[:])
                nm = mpool.tile([P, 1], F32, tag="nm")
                nc.vector.tensor_scalar_mul(nm[:], m[:], -1.0)
                dq = mpool.tile([P, niq], F32, tag="dq")
                for q in range(niq):
                    h, qq = divmod(q * iq, nih)
                    zp = spool.tile([P, iq], F32, tag="sz")
                    mm512(zp[:], lhs, r9[h][:, qq : qq + iq])
                    e = epool.tile([P, iq], F32, tag="e")
                    nc.scalar.activation(
                        out=e[:],
                        in_=zp[:],
                        func=AF.Exp,
                        bias=nm[0:P, 0:1],
                        scale=1.0,
                        accum_out=dq[:, q : q + 1],
                    )
                nc.vector.tensor_reduce(
                    out=dg[:, jci : jci + 1],
                    in_=dq[:],
                    axis=mybir.AxisListType.X,
                    op=ALU.add,
                )
            # one batched Ln per group keeps the ACT Exp table resident longer
            lndg = mpool.tile([P, jcs_per_g], F32, tag="lndg", name="lndg")
            nc.scalar.activation(out=lndg[:], in_=dg[:], func=AF.Ln)
            nc.vector.tensor_tensor(
                out=qcol[:, gs : gs + jcs_per_g], in0=mg[:], in1=lndg[:], op=ALU.add
            )

            # Q splits for this group's columns, interleaved [P, jcs, 3],
            # then one transpose-matmul per 128-j chunk (out base 0)
            ge = gs + jcs_per_g
            qcs = qspl.tile([P, jcs_per_g, 3], BF16, tag="qcs")
            qt1 = qspl.tile([P, jcs_per_g], F32, tag="qt1")
            qt2 = qspl.tile([P, jcs_per_g], F32, tag="qt2")
            nc.vector.tensor_copy(out=qcs[:, :, 0], in_=qcol[:, gs:ge])
            nc.vector.tensor_tensor(
                out=qt1[:], in0=qcol[:, gs:ge], in1=qcs[:, :, 0], op=ALU.subtract
            )
            nc.vector.tensor_copy(out=qcs[:, :, 1], in_=qt1[:])
            nc.vector.tensor_tensor(
                out=qt2[:], in0=qt1[:], in1=qcs[:, :, 1], op=ALU.subtract
            )
            nc.vector.tensor_copy(out=qcs[:, :, 2], in_=qt2[:])

            qsp = opool.tile([3, jg], F32, tag="oz", name="qsp")
            for jci in range(jcs_per_g):
                nc.tensor.matmul(
                    qsp[:, jci * P : (jci + 1) * P],
                    qcs[:, jci, :],
                    i128_s[:],
                    start=True,
                    stop=True,
                )
            nc.vector.tensor_copy(out=src_o[0:3, g * jg : (g + 1) * jg], in_=qsp[:])
            q9p = opool.tile([12, jg], F32, tag="oz", name="q9p")
            mm512(q9p[:], sel_r_o_s[:], src_o[:, g * jg : (g + 1) * jg])
            nc.vector.tensor_copy(out=r9o[:, g * jg : (g + 1) * jg], in_=q9p[:])

            # ---- output pass for this j group ----
            for ich in range(nic // icg):
                ot = outp.tile([P, icg, jg], F32, tag="ot")
                for ici in range(0, icg, 2):
                    ozp = opool.tile([P, 2 * jg], F32, tag="oz")
                    for u in range(2):
                        ic = ich * icg + ici + u
                        nc.tensor.matmul(
                            ozp[:, u * jg : (u + 1) * jg],
                            l9o[:, ic * P : (ic + 1) * P],
                            r9o[:, g * jg : (g + 1) * jg],
                            start=True,
                            stop=True,
                        )
                    nc.scalar.activation(
                        out=ot[:, ici : ici + 2, :],
                        in_=ozp[:],
                        func=AF.Exp,
                    )
                nc.sync.dma_start(
                    out=out.rearrange("(ic p) j -> p ic j", p=P)[
                        :, ich * icg : (ich + 1) * icg, g * jg : (g + 1) * jg
                    ],
                    in_=ot[:],
                )

    nc.compile()
    return nc


# ---------------------------------------------------------------------------


def make_in_maps(scores, n, mode="pair"):
    """Per-core input dicts. Core c -> batch c//2, halves h = c%2."""
    single = mode == "single"
    nj = n if single else n // 2
    nih = n if single else n // 2
    nkc = n // P
    ncores = 1 if single else N_CORES

    cfull = (2 * np.arange(n) + 1 - n).astype(np.float32)
    ch_f, cl_f = _split2(cfull)

    dve_ks = _dve_ks(nkc)
    n_dve = len(dve_ks)

    common = {
        "i128": np.eye(P, dtype=ml_dtypes.bfloat16),
        "sel_l_s": _bf(SEL_L_S),
        "sel_r_s": _bf(SEL_R_S),
        "sel_l_o": _bf(SEL_L_O),
        "sel_r_o": _bf(SEL_R_O),
        "ones3": np.ones((3, P), dtype=ml_dtypes.bfloat16),
        "onesc": np.ones((P, 1), dtype=ml_dtypes.bfloat16),
    }

    in_maps = []
    for c in range(ncores):
        b = 0 if single else c // 2
        h = 0 if single else c % 2
        x = np.asarray(scores[b], dtype=np.float32)
        xh_, xm_, xl_ = _split3(x)
        xch, xcl = _split2(x)
        xcol = np.ascontiguousarray(x.reshape(nkc, P).T)
        xchc = np.ascontiguousarray(xch.reshape(nkc, P).T)
        xclc = np.ascontiguousarray(xcl.reshape(nkc, P).T)
        blhs = np.zeros((P, 3 * nkc), dtype=ml_dtypes.bfloat16)
        blhs[:, 0::3] = 1.0
        blhs[:, 1::3] = xchc
        blhs[:, 2::3] = xclc
        xc2d = np.concatenate([xchc[:, dve_ks], xclc[:, dve_ks]], axis=1)
        assert xc2d.shape[1] == 2 * n_dve
        sl = slice(h * nih, h * nih + nih)
        sj = slice(h * nj, h * nj + nj)
        ones_n = np.ones((1, n), dtype=ml_dtypes.bfloat16)
        xs4 = np.concatenate(
            [xh_[None, :], xm_[None, :], xl_[None, :], ones_n], axis=0
        )
        xsh3 = np.concatenate(
            [xh_[None, sl], xm_[None, sl], xl_[None, sl]], axis=0
        )
        so = slice((1 - h) * nih, (1 - h) * nih + nih) if not single else sl
        xso3 = np.concatenate(
            [xh_[None, so], xm_[None, so], xl_[None, so]], axis=0
        )
        cs3 = np.concatenate(
            [ch_f[None, sj], cl_f[None, sj], np.ones((1, nj), ml_dtypes.bfloat16)],
            axis=0,
        )
        lo_row = (-n + np.arange(P) * (2 * n // P)).astype(np.float32)
        lob = np.tile(lo_row[None, :], (P, 1))
        hib = lob + float(2 * n // P)
        nihc_ = nih // P
        xballh = np.zeros((P, nihc_, 5), dtype=ml_dtypes.bfloat16)
        xballh[:, :, 0] = xchc[:, h * nihc_ : (h + 1) * nihc_]
        xballh[:, :, 1] = xclc[:, h * nihc_ : (h + 1) * nihc_]
        xballh[:, :, 4] = 1.0
        in_maps.append(
            {
                "xs4": xs4,
                "lob": lob,
                "hib": hib,
                "xballh": xballh,
                "xsh3": xsh3,
                "xso3": xso3,
                "cs3": cs3,
                "xcol": xcol,
                "xhc": np.ascontiguousarray(x[sl].reshape(-1, P).T),
                "blhs": blhs,
                "xc2d": np.ascontiguousarray(xc2d),
                **common,
            }
        )
    return in_maps


_NC_CACHE = {}


def _get_nc(n):
    if n not in _NC_CACHE:
        _NC_CACHE[n] = build_nc(n=n, mode="pair", num_devices=N_CORES)
    return _NC_CACHE[n]


def kernel(scores):
    scores = np.asarray(scores, dtype=np.float32)
    b, n = scores.shape
    nj = n // 2
    nc = _get_nc(n)
    in_maps = make_in_maps(scores, n, mode="pair")
    res = run_bass_kernel_spmd(nc, in_maps, list(range(N_CORES)))
    out = np.empty((b, n, n), dtype=np.float32)
    for c in range(N_CORES):
        bb, h = c // 2, c % 2
        out[bb, :, h * nj : (h + 1) * nj] = res.results[c]["out"]
    return out



# revision 16
# speedup vs baseline: 2.0813x; 2.0813x over previous
"""NeuralSort relaxed-permutation kernel for 8 Trainium2 NeuronCores.

out[b, i, j] = softmax_i( s_i * scaling_j - B_i ),  s = -scores[b]
  scaling_j = n - 1 - 2j   =>  z[j,i] = c_j * x_i - B_i  with x = scores[b],
  c_j = -(n - 1 - 2j) = 2j + 1 - n
  B_i = sum_k |x_i - x_k| = x_i*(n - 2*cnt_i) - S + 2*t_i
        cnt_i = #{k: x_k > x_i},  t_i = sum_{k: x_k > x_i} x_k,  S = sum_k x_k

Sharding: core c -> (batch b = c//2, j-half h = c%2). Each core emits the
half-j (n/2) by full-i (n) slab of batch b in [j, i] layout (bf16); the host
transposes while unsharding.

Per-core pipeline (all z matmuls bf16 @ 1 cyc/row, exact via hi/mid/lo
splits):
  P: comparison tiles split across DVE/Pool (is_lt -> {0,1}) and ACT (Sign ->
     {-1,0,1}); PE reduces them with [ones|x_hi|x_lo] stationaries into
     cnt/t rows (PE is primed with dummy matmuls during input load so the
     reduction runs at full p-state). Row results go to column layout via
     tiny PE transposes; combine to -B columns; r1b = signed rank.
  X: AllGather one bf16 payload within the batch pair: [-B bf16 splits
     (column dump) || rank-bucket partial sums as hi/lo bf16 pairs]. The
     -B split rows for the z stacks are scatter-read straight out of the
     payload; x/c rows of the stacks come pre-stacked from the host.
  M: i's are bucketed into 128 rank ranges by r1b (mask tiles on DVE+Pool,
     PE mask-matmul -> per-bucket mean (x_bar, -B_bar)). z at the bucket
     means underestimates each column max by <~40 (z is concave in rank
     space; B_bar >= f(x_bar) by convexity), all the exp shift needs.
  SO (merged stats+output): per 128-j chunk, i on the free axis: rep-z
     matmul + DVE rowmax -> M'_jc just-in-time; z via K=9 matmul into PSUM
     (two 2048-i halves, double buffered); ONE ACT exp(z - M') -> bf16 out
     tile; DVE half-reduces -> D, reciprocal, in-place rescale by 1/D; one
     1 MiB DMA per chunk with 8 KiB contiguous rows. Softmax is
     shift-invariant so exp(z - M')/D is exact regardless of M' slack.
"""

from contextlib import ExitStack

import numpy as np
import ml_dtypes

import concourse.bass as bass
import concourse.tile as tile
from concourse import bacc, mybir
from concourse.bass_utils import run_bass_kernel_spmd

F32 = mybir.dt.float32
BF16 = mybir.dt.bfloat16
AF = mybir.ActivationFunctionType
ALU = mybir.AluOpType

N_CORES = 8
P = 128

# z = sum_k l9[k] * r9[k]; rows ordered so the device-computed -B rows sit
# at partition base 0 (matmul lhsT slices need base 0/32/64) and the
# host-fed x rows are 3-8:
#   l9 = [ 1,   1,   1,  chi, clo, chi, clo, chi, clo]
#   r9 = [-Bh, -Bm, -Bl, xh,  xh,  xm,  xm,  xl,  xl]


def _bf(x):
    return np.asarray(x, dtype=ml_dtypes.bfloat16)


def _split3(x):
    x = np.asarray(x, dtype=np.float32)
    h = _bf(x)
    r = x - h.astype(np.float32)
    m = _bf(r)
    l = _bf(r - m.astype(np.float32))
    return h, m, l


def _split2(x):
    x = np.asarray(x, dtype=np.float32)
    h = _bf(x)
    l = _bf(x - h.astype(np.float32))
    return h, l


def _cmp_engines(nkc):
    """Comparison-chunk engine assignment: 'd' DVE is_lt, 'a' ACT Sign.
    (Pool cannot run TensorScalarPtr on TRN2.) Rates ~1127/1892 ns per
    chunk -> 5:3 mix keeps both generators finishing together while PE
    (852 ns/chunk ramped) stays the binding resource."""
    pat = ["d", "a", "d", "d", "a", "d", "a", "d"]
    eng = [pat[k % len(pat)] for k in range(nkc)]
    if nkc >= 2 and eng[-1] == "a":
        eng[-1], eng[-2] = eng[-2], eng[-1]
    return eng


def _islt_ks(nkc):
    """Chunks whose comparisons use is_lt (DVE+Pool); rest use ACT Sign."""
    eng = _cmp_engines(nkc)
    return [k for k in range(nkc) if eng[k] != "a"]


def build_nc(n=4096, mode="pair", num_devices=N_CORES):
    """mode: "pair" (8-core, AllGather within batch pairs); "single" (1-core
    debug: full j/i ranges, no collective); "timing" (pair shapes,
    collective replaced by local copies -- for the 1-core timeline model)."""
    single = mode == "single"
    use_collective = mode == "pair"
    nj = n if single else n // 2    # output columns (j) per core
    nih = n if single else n // 2   # i-range whose B this core computes
    nkc = n // P                    # k-chunks in the comparison pass
    njc = nj // P                   # 128-wide j-chunks
    nihc = nih // P                 # 128-wide i-chunks of the own half
    ih = n // 2                     # i-half width for the SO z tiles

    nc = bacc.Bacc(
        "TRN2", target_bir_lowering=False, debug=False, num_devices=num_devices
    )

    def din(name, shape, dt=F32):
        return nc.dram_tensor(name, shape, dt, kind="ExternalInput").ap()

    eng_ks = _cmp_engines(nkc)
    islt_ks = _islt_ks(nkc)
    n_islt = len(islt_ks)

    # packed small constants: one bf16 blob + one f32 blob, sliced on SBUF
    ob_blhs, ob_xc2d = 0, 3 * nkc
    ob_i128 = ob_xc2d + 2 * n_islt
    ob_onesc = ob_i128 + P
    wb = ob_onesc + 1
    of_xcol, of_xhc = 0, nkc
    of_lob = of_xhc + nihc
    of_hib = of_lob + P
    of_i6 = of_hib + P
    wf = of_i6 + 6

    xbf = din("xbf", [P, nih], F32)        # x of own i-half, broadcast 128x
    l9full = din("l9full", [9, nj], BF16)  # host-stacked z lhs rows
    pkf = din("pkf", [P, wf], F32)
    pkb = din("pkb", [P, wb], BF16)
    xr6 = din("xr6", [6, nih], BF16)       # r9 rows 0-5, own half
    xr6o = din("xr6o", [6, nih], BF16)     # r9 rows 0-5, partner half
    xballh = din("xballh", [P, nihc, 5], BF16)  # own cols [xch,xcl,0,0,1]

    # output in [j, i] layout, bf16; host transposes while unsharding
    out = nc.dram_tensor("out", [nj, n], BF16, kind="ExternalOutput").ap()

    # exchange payload (bf16): the -B splits in (s, t, p) order
    nsp = 3 * nihc                  # -B split columns per partition
    npay = P * nsp
    bh_dram = nc.dram_tensor("bh_dram", [1, npay], BF16).ap()
    nhalves = 1 if single else 2
    bfull_dram = nc.dram_tensor("bfull_dram", [nhalves, npay], BF16).ap()
    groups = [[2 * p, 2 * p + 1] for p in range(max(1, num_devices // 2))]

    def mm512(out_ap, lhsT, rhs, start=True, stop=True):
        """matmul with the moving dim split into <=512-column chunks."""
        nfree = rhs.shape[-1]
        assert out_ap.shape[-1] == nfree
        for o in range(0, nfree, 512):
            e = min(o + 512, nfree)
            nc.tensor.matmul(
                out_ap[..., o:e], lhsT, rhs[..., o:e], start=start, stop=stop
            )

    with tile.TileContext(nc) as tc, ExitStack() as ctx:
        cpool = ctx.enter_context(tc.tile_pool(name="consts", bufs=1))

        def load(pool, ap_dram, shape, dt, name):
            t = pool.tile(shape, dt, tag=name)
            nc.sync.dma_start(out=t[:], in_=ap_dram)
            return t

        # input loads, critical-path first (xb halved so chunk-0 comparisons
        # can start on the first half)
        xb = cpool.tile([P, nih], F32, tag="xb")
        nc.sync.dma_start(out=xb[:, 0 : nih // 2], in_=xbf[:, 0 : nih // 2])
        pkf_s = load(cpool, pkf, [P, wf], F32, "pkf")
        nc.sync.dma_start(out=xb[:, nih // 2 : nih], in_=xbf[:, nih // 2 : nih])
        pkb_s = load(cpool, pkb, [P, wb], BF16, "pkb")
        l9 = load(cpool, l9full, [9, nj], BF16, "l9")
        r9 = [
            cpool.tile([9, nih], BF16, tag=f"r9_{h}", name=f"r9_{h}")
            for h in range(nhalves)
        ]
        nc.sync.dma_start(out=r9[0][3:9, :], in_=xr6)
        if nhalves == 2:
            nc.sync.dma_start(out=r9[1][3:9, :], in_=xr6o)
        rep9 = cpool.tile([9, P], BF16, tag="rep9")
        nmcol = cpool.tile([P, n // P if single else n // 2 // P], F32,
                           tag="nmcol")

        xcol_s = pkf_s[:, of_xcol : of_xcol + nkc]
        xhc_s = pkf_s[:, of_xhc : of_xhc + nihc]
        lob_s = pkf_s[:, of_lob : of_lob + P]
        hib_s = pkf_s[:, of_hib : of_hib + P]
        i6f_s = pkf_s[0:6, of_i6 : of_i6 + 6]
        blhs_s = pkb_s[:, ob_blhs : ob_blhs + 3 * nkc]
        xc2d_s = pkb_s[:, ob_xc2d : ob_xc2d + 2 * n_islt]
        i128_s = pkb_s[:, ob_i128 : ob_i128 + P]
        onesc_s = pkb_s[:, ob_onesc : ob_onesc + 1]

        # ---- PE warm-up: keep the p-state ramp going while inputs load,
        # and preload the ACT function table with a dummy Sign ----
        wt = cpool.tile([P, 512], BF16, tag="wt")
        with (
            tc.tile_pool(name="warmp", bufs=1, space="PSUM") as wpp,
        ):
            nc.vector.memset(wt[:], 1.0)
            wsg = cpool.tile([1, 1], BF16, tag="wsg")
            nc.scalar.activation(out=wsg[:], in_=wt[0:1, 0:1], func=AF.Sign)
            wps = wpp.tile([P, 512], F32)
            for _ in range(10):
                nc.tensor.matmul(
                    wps[:], wt[:, 0:P], wt[:], start=True, stop=True
                )

        with tc.tile_pool(name="prep", bufs=1) as pp_s:
            # comparison pass across DVE/Pool (is_lt -> {0,1}) and ACT
            # (Sign -> {-1,0,1}); PE drains g tiles in program order. For
            # the is_lt set, sum_k |x_i-x_k| = x_i*(nD - 2*cntD) - SD +
            # 2*tD; for the ACT set it's x_i*sgnS - tS. Ties vanish either
            # way.
            nxcol = pp_s.tile([P, nkc], F32, tag="nxcol")
            nc.vector.tensor_scalar_mul(nxcol[:], xcol_s, -1.0)
            b3 = pp_s.tile([3, nih], F32, tag="b3")
            b3s = pp_s.tile([3, nih], F32, tag="b3s")
            with (
                tc.tile_pool(name="bp", bufs=1, space="PSUM") as bp,
                tc.tile_pool(name="gp", bufs=6) as gp,
            ):
                bpsum = bp.tile([3, nih], F32)
                bpsum2 = bp.tile([3, nih], F32)
                nlt_seen = nact_seen = 0
                for k in range(nkc):
                    g = gp.tile([P, nih], BF16, tag="g")
                    e = eng_ks[k]
                    if e != "a":
                        nlt_seen += 1
                        veng = nc.vector if e == "d" else nc.gpsimd
                        spans = (
                            [(0, nih // 2), (nih // 2, nih)]
                            if nlt_seen <= 2
                            else [(0, nih)]
                        )
                        for a0, a1 in spans:
                            veng.tensor_scalar(
                                out=g[:, a0:a1],
                                in0=xb[:, a0:a1],
                                scalar1=xcol_s[:, k : k + 1],
                                scalar2=None,
                                op0=ALU.is_lt,
                            )
                        mm512(
                            bpsum[:],
                            blhs_s[:, 3 * k : 3 * k + 3],
                            g[:],
                            start=(nlt_seen == 1),
                            stop=(nlt_seen == n_islt),
                        )
                    else:
                        nact_seen += 1
                        nc.scalar.activation(
                            out=g[:],
                            in_=xb[:],
                            func=AF.Sign,
                            bias=nxcol[0:P, k : k + 1],
                        )
                        mm512(
                            bpsum2[:],
                            blhs_s[:, 3 * k : 3 * k + 3],
                            g[:],
                            start=(nact_seen == 1),
                            stop=(nact_seen == nkc - n_islt),
                        )
                # drain both accumulators, split across DVE and ACT so the
                # copies overlap (compute APs must start at partition 0)
                hw2 = nih // 2
                nc.vector.tensor_copy(out=b3[:, 0:hw2], in_=bpsum[:, 0:hw2])
                nc.scalar.activation(
                    out=b3[:, hw2:nih], in_=bpsum[:, hw2:nih], func=AF.Copy
                )
                nc.vector.tensor_copy(out=b3s[:, 0:hw2], in_=bpsum2[:, 0:hw2])
                nc.scalar.activation(
                    out=b3s[:, hw2:nih], in_=bpsum2[:, hw2:nih], func=AF.Copy
                )

            # rows -> columns via tiny PE transposes (exact data movement)
            bc_all = pp_s.tile([P, nihc, 6], F32, tag="bc_all")
            with tc.tile_pool(name="tp", bufs=1, space="PSUM") as tp:
                bc_ps = tp.tile([P, nihc, 6], F32)
                for ch in range(nihc):
                    nc.tensor.transpose(
                        bc_ps[:, ch, 0:3],
                        b3[:, ch * P : (ch + 1) * P],
                        i6f_s[0:3, 0:3],
                    )
                    nc.tensor.transpose(
                        bc_ps[:, ch, 3:6],
                        b3s[:, ch * P : (ch + 1) * P],
                        i6f_s[0:3, 0:3],
                    )
                nc.vector.tensor_copy(out=bc_all[:], in_=bc_ps[:])
            cntc = bc_all[:, :, 0]
            thc = bc_all[:, :, 1]
            tlc = bc_all[:, :, 2]
            sgnc = bc_all[:, :, 3]
            tshc = bc_all[:, :, 4]
            tslc = bc_all[:, :, 5]

            # +SD (sum of x over the is_lt-chunk k's)
            spos = pp_s.tile([1, 1], F32, tag="spos")
            with tc.tile_pool(name="sp", bufs=1, space="PSUM") as sp:
                sxp = sp.tile([1, 2 * n_islt], F32)
                nc.tensor.matmul(sxp[:], onesc_s, xc2d_s, start=True, stop=True)
                nc.vector.tensor_reduce(
                    out=spos[:], in_=sxp[:], axis=mybir.AxisListType.X, op=ALU.add
                )
            sposc = pp_s.tile([P, 1], F32, tag="sposc")
            nc.gpsimd.partition_broadcast(sposc[:], spos[0:1, 0:1])

            # -B = -x*(nD - 2*cntD + sgnS) + SD - 2*(tDh+tDl) + (tSh+tSl)
            nD = float(n_islt * P)
            r1 = pp_s.tile([P, nihc], F32, tag="r1")
            nc.vector.tensor_scalar(
                out=r1[:],
                in0=cntc,
                scalar1=-2.0,
                scalar2=nD,
                op0=ALU.mult,
                op1=ALU.add,
            )
            r1b = pp_s.tile([P, nihc], F32, tag="r1b")
            nc.vector.tensor_tensor(out=r1b[:], in0=r1[:], in1=sgnc, op=ALU.add)
            r2n = pp_s.tile([P, nihc], F32, tag="r2n")
            nc.vector.scalar_tensor_tensor(
                out=r2n[:], in0=xhc_s, scalar=-1.0, in1=r1b[:],
                op0=ALU.mult, op1=ALU.mult,
            )
            tt = pp_s.tile([P, nihc], F32, tag="tt")
            nc.vector.tensor_tensor(out=tt[:], in0=thc, in1=tlc, op=ALU.add)
            u1 = pp_s.tile([P, nihc], F32, tag="u1")
            nc.vector.scalar_tensor_tensor(
                out=u1[:], in0=tt[:], scalar=-2.0, in1=r2n[:],
                op0=ALU.mult, op1=ALU.add,
            )
            tts = pp_s.tile([P, nihc], F32, tag="tts")
            nc.vector.tensor_tensor(out=tts[:], in0=tshc, in1=tslc, op=ALU.add)
            u2 = pp_s.tile([P, nihc], F32, tag="u2")
            nc.vector.tensor_tensor(out=u2[:], in0=u1[:], in1=tts[:], op=ALU.add)
            nbhalfc = pp_s.tile([P, nihc], F32, tag="nbhalfc")
            nc.vector.tensor_scalar(
                out=nbhalfc[:],
                in0=u2[:],
                scalar1=sposc[:, 0:1],
                scalar2=None,
                op0=ALU.add,
            )

            # -B bf16 splits, chunk-major [P, t, s] so each chunk's three
            # split columns sit adjacent for the PE row-transposes below
            nbsh = pp_s.tile([P, nihc, 3], BF16, tag="nbsh")
            sp0 = nbsh[:, :, 0]
            sp1 = nbsh[:, :, 1]
            sp2 = nbsh[:, :, 2]
            cs_t1 = pp_s.tile([P, nihc], F32, tag="cs_t1")
            cs_t2 = pp_s.tile([P, nihc], F32, tag="cs_t2")
            nc.vector.tensor_copy(out=sp0, in_=nbhalfc[:])
            nc.vector.tensor_tensor(out=cs_t1[:], in0=nbhalfc[:], in1=sp0,
                                    op=ALU.subtract)
            nc.vector.tensor_copy(out=sp1, in_=cs_t1[:])
            nc.vector.tensor_tensor(out=cs_t2[:], in0=cs_t1[:], in1=sp1,
                                    op=ALU.subtract)
            nc.vector.tensor_copy(out=sp2, in_=cs_t2[:])

            # ---- own-half rank-bucket partial sums (pre-exchange) ----
            xballh_s = load(pp_s, xballh, [P, nihc, 5], BF16, "xballh")
            nc.vector.tensor_copy(out=xballh_s[:, :, 2], in_=nbsh[:, :, 0])
            nc.vector.tensor_copy(out=xballh_s[:, :, 3], in_=nbsh[:, :, 1])
            reps_own = pp_s.tile([P, 5], F32, tag="reps_own")
            with (
                tc.tile_pool(name="repp0", bufs=1, space="PSUM") as repp0,
                tc.tile_pool(name="mkp0", bufs=4) as mkp0,
            ):
                repso_p = repp0.tile([P, 5], F32)
                for ch in range(nihc):
                    meng = nc.vector
                    m1 = mkp0.tile([P, P], BF16, tag="m1")
                    meng.tensor_scalar(
                        out=m1[:],
                        in0=lob_s,
                        scalar1=r1b[:, ch : ch + 1],
                        scalar2=None,
                        op0=ALU.is_le,
                    )
                    msk = mkp0.tile([P, P], BF16, tag="msk")
                    meng.scalar_tensor_tensor(
                        out=msk[:],
                        in0=hib_s,
                        scalar=r1b[:, ch : ch + 1],
                        in1=m1[:],
                        op0=ALU.is_gt,
                        op1=ALU.mult,
                    )
                    nc.tensor.matmul(
                        repso_p[:],
                        msk[:],
                        xballh_s[:, ch, :],
                        start=(ch == 0),
                        stop=(ch == nihc - 1),
                    )
                nc.vector.tensor_copy(out=reps_own[:], in_=repso_p[:])
            # ---- -B rows straight into r9[0] via tiny PE transposes (PE
            # and ACT are idle here; skips the slow element-scatter DMA),
            # then one contiguous DMA publishes them as the payload ----
            with tc.tile_pool(name="btp", bufs=1, space="PSUM") as btp:
                btr = btp.tile([3, nih], BF16)
                for ch in range(nihc):
                    nc.tensor.transpose(
                        btr[:, ch * P : (ch + 1) * P], nbsh[:, ch, :], i128_s
                    )
                nc.scalar.activation(out=r9[0][0:3, :], in_=btr[:], func=AF.Copy)
            nc.sync.dma_start(
                out=bh_dram[0, 0 : P * nsp].rearrange("(s i) -> s i", s=3),
                in_=r9[0][0:3, :],
            )
            if use_collective:
                nc.gpsimd.collective_compute(
                    "AllGather",
                    ALU.bypass,
                    replica_groups=groups,
                    ins=[bh_dram],
                    outs=[bfull_dram],
                )
            else:
                for hh in range(nhalves):
                    nc.sync.dma_start(out=bfull_dram[hh : hh + 1, :], in_=bh_dram)

            if nhalves == 2:
                # AllGather slots are by group position, so slot 1 is this
                # very core on odd ranks. Recover the partner rows position-
                # free: partner = (slab0 + slab1) - own (exact: within-row
                # magnitudes are homogeneous bf16 values, so the f32 sums
                # round-trip exactly)
                s01 = pp_s.tile([3, 2, nih], BF16, tag="s01")
                nc.sync.dma_start(
                    out=s01[:],
                    in_=bfull_dram[0:2, 0 : P * nsp].rearrange(
                        "h (s i) -> s h i", s=3
                    ),
                )
                ssum = pp_s.tile([3, nih], F32, tag="ssum2")
                nc.vector.tensor_tensor(
                    out=ssum[:], in0=s01[:, 0, :], in1=s01[:, 1, :], op=ALU.add
                )
                nc.vector.tensor_tensor(
                    out=r9[1][0:3, :], in0=ssum[:], in1=r9[0][0:3, :],
                    op=ALU.subtract,
                )

            # ---- rank-bucket representatives -> rep9 stack ----
            # OWN-half reps only: the top ranks of every column are never
            # all in the partner half (P ~ 2^-30), so the own-half bucket
            # max underestimates each column max by only a few more units
            # than the full-data version -- still far inside the exp(88)
            # bf16 budget, and softmax shift-invariance keeps the result
            # exact. This takes the whole M' pipeline off the exchange
            # critical path.
            # reps rows: [sum xh, sum xl, sum -Bh, sum -Bm, count]
            reps = reps_own
            cnt1 = pp_s.tile([P, 1], F32, tag="cnt1")
            nc.vector.tensor_scalar_max(cnt1[:], reps[:, 4:5], 1.0)
            rc = pp_s.tile([P, 1], F32, tag="rc")
            nc.vector.reciprocal(rc[:], cnt1[:])
            repx = pp_s.tile([P, 1], F32, tag="repx")
            nc.vector.tensor_tensor(
                out=repx[:], in0=reps[:, 0:1], in1=reps[:, 1:2], op=ALU.add
            )
            nc.vector.tensor_tensor(
                out=repx[:], in0=repx[:], in1=rc[:], op=ALU.mult
            )
            repb = pp_s.tile([P, 1], F32, tag="repb")  # mean of -B
            nc.vector.tensor_tensor(
                out=repb[:], in0=reps[:, 2:3], in1=reps[:, 3:4], op=ALU.add
            )
            nc.vector.tensor_tensor(
                out=repb[:], in0=repb[:], in1=rc[:], op=ALU.mult
            )
            # empty bucket -> push its line to -inf
            iz = pp_s.tile([P, 1], F32, tag="iz")
            nc.vector.tensor_scalar(
                out=iz[:], in0=reps[:, 4:5], scalar1=0.5, scalar2=None,
                op0=ALU.is_le,
            )
            nc.vector.scalar_tensor_tensor(
                out=repb[:], in0=iz[:], scalar=-1e30, in1=repb[:],
                op0=ALU.mult, op1=ALU.add,
            )
            # rep9 columns pre-transpose: [nBh2,nBm2,0,xh2,xh2,xl2,xl2,0,0]
            rs9 = pp_s.tile([P, 9], BF16, tag="rs9")
            rtmp = pp_s.tile([P, 1], F32, tag="rep_rt")
            nc.vector.tensor_copy(out=rs9[:, 0:1], in_=repb[:])
            nc.vector.tensor_tensor(
                out=rtmp[:], in0=repb[:], in1=rs9[:, 0:1], op=ALU.subtract
            )
            nc.vector.tensor_copy(out=rs9[:, 1:2], in_=rtmp[:])
            nc.vector.memset(rs9[:, 2:3], 0.0)
            nc.vector.tensor_copy(out=rs9[:, 3:4], in_=repx[:])
            nc.vector.tensor_copy(out=rs9[:, 4:5], in_=rs9[:, 3:4])
            nc.vector.tensor_tensor(
                out=rtmp[:], in0=repx[:], in1=rs9[:, 3:4], op=ALU.subtract
            )
            nc.vector.tensor_copy(out=rs9[:, 5:6], in_=rtmp[:])
            nc.vector.tensor_copy(out=rs9[:, 6:7], in_=rs9[:, 5:6])
            nc.vector.memset(rs9[:, 7:9], 0.0)
            with tc.tile_pool(name="repp", bufs=1, space="PSUM") as repp:
                p9r = repp.tile([9, P], F32)
                nc.tensor.matmul(p9r[:], rs9[:], i128_s, start=True, stop=True)
                nc.vector.tensor_copy(out=rep9[:], in_=p9r[:])

            # ---- M' for every j-chunk upfront (needs only rep9 + l9, so
            # this overlaps the collective): z at the 128 bucket reps per
            # chunk, one batched DVE max-reduce, negate ----
            with tc.tile_pool(name="mrp", bufs=1, space="PSUM") as mrp:
                zrep = mrp.tile([P, njc, P], F32)
                for jc in range(njc):
                    nc.tensor.matmul(
                        zrep[:, jc, :],
                        l9[:, jc * P : (jc + 1) * P],
                        rep9[:],
                        start=True,
                        stop=True,
                    )
                mcol = pp_s.tile([P, njc], F32, tag="mcol")
                nsp0 = min(4, njc)
                nc.vector.tensor_reduce(
                    out=mcol[:, 0:nsp0], in_=zrep[:, 0:nsp0, :],
                    axis=mybir.AxisListType.X, op=ALU.max,
                )
                nc.vector.tensor_scalar_mul(
                    nmcol[:, 0:nsp0], mcol[:, 0:nsp0], -1.0
                )
                if njc > nsp0:
                    nc.vector.tensor_reduce(
                        out=mcol[:, nsp0:njc], in_=zrep[:, nsp0:njc, :],
                        axis=mybir.AxisListType.X, op=ALU.max,
                    )
                    nc.vector.tensor_scalar_mul(
                        nmcol[:, nsp0:njc], mcol[:, nsp0:njc], -1.0
                    )

        # ---------------- Phase SO: merged softmax+output per j-chunk -------
        # software-pipelined half-streams: the own-half (q=0) exp of chunk c
        # is issued before the partner-half (q=1) exp of chunk c-1, so the
        # ACT stream starts as soon as r9[0] lands -- before the collective
        # delivers r9[1].
        spool = ctx.enter_context(tc.tile_pool(name="sz", bufs=2, space="PSUM"))
        dpool = ctx.enter_context(tc.tile_pool(name="dd", bufs=6))
        outp = ctx.enter_context(tc.tile_pool(name="outp", bufs=4))
        nhi = n // ih  # i-halves per chunk

        def z_half(zp, lhs, q):
            h, qq = divmod(q * ih, nih)
            o = 0
            while o < ih:
                hh, qo = h, qq + o
                if qo >= nih:
                    hh, qo = h + 1, qo - nih
                e = min(qo + 512, nih) - qo
                nc.tensor.matmul(
                    zp[:, o : o + e],
                    lhs,
                    r9[hh][:, qo : qo + e],
                    start=True,
                    stop=True,
                )
                o += e

        def finalize(st):
            ot, dq = st["ot"], st["dq"]
            dsum = dpool.tile([P, 1], F32, tag="dsum")
            nc.vector.tensor_tensor(
                out=dsum[:], in0=dq[:, 0:1], in1=dq[:, 1:2], op=ALU.add
            )
            rcp = dpool.tile([P, 1], F32, tag="rcp")
            nc.vector.reciprocal(rcp[:], dsum[:])
            nc.vector.tensor_scalar(
                out=ot[:],
                in0=ot[:],
                scalar1=rcp[:, 0:1],
                scalar2=None,
                op0=ALU.mult,
            )
            nc.sync.dma_start(
                out=out.rearrange("(jc p) i -> p jc i", p=P)[
                    :, st["jc"], :
                ],
                in_=ot[:],
            )

        prev = None
        for jc in range(njc):
            lhs = l9[:, jc * P : (jc + 1) * P]
            st = {
                "jc": jc,
                "ot": outp.tile([P, n], BF16, tag="ot", name="ot"),
                "dq": dpool.tile([P, 2], F32, tag="dq", name="dq"),
                "lhs": lhs,
            }
            zp = spool.tile([P, ih], F32, tag="sz")
            z_half(zp, lhs, 0)
            nc.scalar.activation(
                out=st["ot"][:, 0:ih],
                in_=zp[:],
                func=AF.Exp,
                bias=nmcol[0:P, jc : jc + 1],
                scale=1.0,
                accum_out=st["dq"][:, 0:1],
            )
            if prev is not None:
                zp1 = spool.tile([P, ih], F32, tag="sz")
                z_half(zp1, prev["lhs"], 1)
                nc.scalar.activation(
                    out=prev["ot"][:, ih : 2 * ih],
                    in_=zp1[:],
                    func=AF.Exp,
                    bias=nmcol[0:P, prev["jc"] : prev["jc"] + 1],
                    scale=1.0,
                    accum_out=prev["dq"][:, 1:2],
                )
                finalize(prev)
            prev = st
        zp1 = spool.tile([P, ih], F32, tag="sz")
        z_half(zp1, prev["lhs"], 1)
        nc.scalar.activation(
            out=prev["ot"][:, ih : 2 * ih],
            in_=zp1[:],
            func=AF.Exp,
            bias=nmcol[0:P, prev["jc"] : prev["jc"] + 1],
            scale=1.0,
            accum_out=prev["dq"][:, 1:2],
        )
        finalize(prev)

    nc.compile()
    return nc


# ---------------------------------------------------------------------------


def make_in_maps(scores, n, mode="pair"):
    """Per-core input dicts. Core c -> batch c//2, halves h = c%2."""
    single = mode == "single"
    nj = n if single else n // 2
    nih = n if single else n // 2
    nkc = n // P
    nihc = nih // P
    ncores = 1 if single else N_CORES

    cfull = (2 * np.arange(n) + 1 - n).astype(np.float32)
    ch_f, cl_f = _split2(cfull)

    islt_ks = _islt_ks(nkc)
    n_islt = len(islt_ks)

    in_maps = []
    for c in range(ncores):
        b = 0 if single else c // 2
        h = 0 if single else c % 2
        x = np.asarray(scores[b], dtype=np.float32)
        xh_, xm_, xl_ = _split3(x)
        xch, xcl = _split2(x)
        xcol = np.ascontiguousarray(x.reshape(nkc, P).T).astype(np.float32)
        xchc = np.ascontiguousarray(xch.reshape(nkc, P).T)
        xclc = np.ascontiguousarray(xcl.reshape(nkc, P).T)
        blhs = np.zeros((P, 3 * nkc), dtype=ml_dtypes.bfloat16)
        blhs[:, 0::3] = 1.0
        blhs[:, 1::3] = xchc
        blhs[:, 2::3] = xclc
        xc2d = np.concatenate([xchc[:, islt_ks], xclc[:, islt_ks]], axis=1)
        assert xc2d.shape[1] == 2 * n_islt
        sl = slice(h * nih, h * nih + nih)
        sj = slice(h * nj, h * nj + nj)
        so = slice((1 - h) * nih, (1 - h) * nih + nih) if not single else sl

        def xr6_of(s):
            return np.stack(
                [xh_[s], xh_[s], xm_[s], xm_[s], xl_[s], xl_[s]], axis=0
            )

        ones_j = np.ones((3, nj), dtype=ml_dtypes.bfloat16)
        l9full = np.concatenate(
            [
                ones_j,
                ch_f[None, sj], cl_f[None, sj],
                ch_f[None, sj], cl_f[None, sj],
                ch_f[None, sj], cl_f[None, sj],
            ],
            axis=0,
        )
        lo_row = (-n + np.arange(P) * (2 * n // P)).astype(np.float32)
        lob = np.tile(lo_row[None, :], (P, 1))
        hib = lob + float(2 * n // P)
        xballh = np.zeros((P, nihc, 5), dtype=ml_dtypes.bfloat16)
        xballh[:, :, 0] = xchc[:, h * nihc : (h + 1) * nihc]
        xballh[:, :, 1] = xclc[:, h * nihc : (h + 1) * nihc]
        xballh[:, :, 4] = 1.0

        wb = 3 * nkc + 2 * n_islt + P + 1
        pkb = np.zeros((P, wb), dtype=ml_dtypes.bfloat16)
        o = 0
        pkb[:, o : o + 3 * nkc] = blhs
        o += 3 * nkc
        pkb[:, o : o + 2 * n_islt] = xc2d
        o += 2 * n_islt
        pkb[:, o : o + P] = np.eye(P, dtype=ml_dtypes.bfloat16)
        o += P
        pkb[:, o] = 1.0  # onesc
        o += 1
        assert o == wb

        wf = nkc + nihc + P + P + 6
        pkf = np.zeros((P, wf), dtype=np.float32)
        o = 0
        pkf[:, o : o + nkc] = xcol
        o += nkc
        pkf[:, o : o + nihc] = np.ascontiguousarray(x[sl].reshape(-1, P).T)
        o += nihc
        pkf[:, o : o + P] = lob
        o += P
        pkf[:, o : o + P] = hib
        o += P
        pkf[0:6, o : o + 6] = np.eye(6, dtype=np.float32)
        o += 6
        assert o == wf

        in_maps.append(
            {
                "xbf": np.tile(x[sl][None, :], (P, 1)),
                "l9full": l9full,
                "pkf": pkf,
                "pkb": pkb,
                "xr6": xr6_of(sl),
                "xr6o": xr6_of(so),
                "xballh": xballh,
            }
        )
    return in_maps


_NC_CACHE = {}


def _get_nc(n):
    if n not in _NC_CACHE:
        _NC_CACHE[n] = build_nc(n=n, mode="pair", num_devices=N_CORES)
    return _NC_CACHE[n]


def kernel(scores):
    scores = np.asarray(scores, dtype=np.float32)
    b, n = scores.shape
    nj = n // 2
    nih = n // 2
    nc = _get_nc(n)
    in_maps = make_in_maps(scores, n, mode="pair")
    res = run_bass_kernel_spmd(nc, in_maps, list(range(N_CORES)))
    out = np.empty((b, n, n), dtype=np.float32)
    for c in range(N_CORES):
        bb, h = c // 2, c % 2
        odev = np.asarray(res.results[c]["out"], dtype=np.float32)  # [nj, n]
        # odev columns: [own half (i in h-half) | partner half]
        out[bb, h * nih : (h + 1) * nih, h * nj : (h + 1) * nj] = odev[
            :, 0:nih
        ].T
        out[bb, (1 - h) * nih : (2 - h) * nih, h * nj : (h + 1) * nj] = odev[
            :, nih : 2 * nih
        ].T
    return out


# revision 22
# speedup vs baseline: 2.2661x; 1.0888x over previous
"""NeuralSort relaxed-permutation kernel for 8 Trainium2 NeuronCores.

out[b, i, j] = softmax_i( s_i * scaling_j - B_i ),  s = -scores[b]
  scaling_j = n - 1 - 2j   =>  z[j,i] = c_j * x_i - B_i  with x = scores[b],
  c_j = -(n - 1 - 2j) = 2j + 1 - n
  B_i = sum_k |x_i - x_k| = x_i*(n - 2*cnt_i) - S + 2*t_i
        cnt_i = #{k: x_k > x_i},  t_i = sum_{k: x_k > x_i} x_k,  S = sum_k x_k

Sharding: core c -> (batch b = c//2, j-half h = c%2). Each core emits the
half-j (n/2) by full-i (n) slab of batch b in [j, i] layout (bf16); the host
transposes while unsharding.

Per-core pipeline (all z matmuls bf16 @ 1 cyc/row, exact via hi/mid/lo
splits):
  P: comparison tiles split across DVE/Pool (is_lt -> {0,1}) and ACT (Sign ->
     {-1,0,1}); PE reduces them with [ones|x_hi|x_lo] stationaries into
     cnt/t rows (PE is primed with dummy matmuls during input load so the
     reduction runs at full p-state). Row results go to column layout via
     tiny PE transposes; combine to -B columns; r1b = signed rank.
  X: AllGather one bf16 payload within the batch pair: [-B bf16 splits
     (column dump) || rank-bucket partial sums as hi/lo bf16 pairs]. The
     -B split rows for the z stacks are scatter-read straight out of the
     payload; x/c rows of the stacks come pre-stacked from the host.
  M: i's are bucketed into 128 rank ranges by r1b (mask tiles on DVE+Pool,
     PE mask-matmul -> per-bucket mean (x_bar, -B_bar)). z at the bucket
     means underestimates each column max by <~40 (z is concave in rank
     space; B_bar >= f(x_bar) by convexity), all the exp shift needs.
  SO (merged stats+output): per 128-j chunk, i on the free axis: rep-z
     matmul + DVE rowmax -> M'_jc just-in-time; z via K=9 matmul into PSUM
     (two 2048-i halves, double buffered); ONE ACT exp(z - M') -> bf16 out
     tile; DVE half-reduces -> D, reciprocal, in-place rescale by 1/D; one
     1 MiB DMA per chunk with 8 KiB contiguous rows. Softmax is
     shift-invariant so exp(z - M')/D is exact regardless of M' slack.
"""

from contextlib import ExitStack

import numpy as np
import ml_dtypes

import concourse.bass as bass
import concourse.tile as tile
from concourse import bacc, mybir
from concourse.bass_utils import run_bass_kernel_spmd

F32 = mybir.dt.float32
BF16 = mybir.dt.bfloat16
AF = mybir.ActivationFunctionType
ALU = mybir.AluOpType

N_CORES = 8
P = 128

# z = sum_k l9[k] * r9[k]; rows ordered so the device-computed -B rows sit
# at partition base 0 (matmul lhsT slices need base 0/32/64) and the
# host-fed x rows are 3-8:
#   l9 = [ 1,   1,   1,  chi, clo, chi, clo, chi, clo]
#   r9 = [-Bh, -Bm, -Bl, xh,  xh,  xm,  xm,  xl,  xl]


def _bf(x):
    return np.asarray(x, dtype=ml_dtypes.bfloat16)


def _split3(x):
    x = np.asarray(x, dtype=np.float32)
    h = _bf(x)
    r = x - h.astype(np.float32)
    m = _bf(r)
    l = _bf(r - m.astype(np.float32))
    return h, m, l


def _split2(x):
    x = np.asarray(x, dtype=np.float32)
    h = _bf(x)
    l = _bf(x - h.astype(np.float32))
    return h, l


def _cmp_engines(nkc):
    """Comparison-chunk engine assignment: 'd' DVE is_lt, 'a' ACT Sign.
    (Pool cannot run TensorScalarPtr on TRN2.) Rates ~1127/1892 ns per
    chunk -> 5:3 mix keeps both generators finishing together while PE
    (852 ns/chunk ramped) stays the binding resource."""
    pat = ["d", "a", "d", "d", "a", "d", "a", "d"]
    eng = [pat[k % len(pat)] for k in range(nkc)]
    if nkc >= 2 and eng[-1] == "a":
        eng[-1], eng[-2] = eng[-2], eng[-1]
    return eng


def _islt_ks(nkc):
    """Chunks whose comparisons use is_lt (DVE+Pool); rest use ACT Sign."""
    eng = _cmp_engines(nkc)
    return [k for k in range(nkc) if eng[k] != "a"]


def build_nc(n=4096, mode="pair", num_devices=N_CORES):
    """mode: "pair" (8-core, AllGather within batch pairs); "single" (1-core
    debug: full j/i ranges, no collective); "timing" (pair shapes,
    collective replaced by local copies -- for the 1-core timeline model)."""
    single = mode == "single"
    use_collective = mode == "pair"
    nj = n if single else n // 2    # output columns (j) per core
    nih = n if single else n // 2   # i-range whose B this core computes
    nkc = n // P                    # k-chunks in the comparison pass
    njc = nj // P                   # 128-wide j-chunks
    nihc = nih // P                 # 128-wide i-chunks of the own half
    ih = n // 2                     # i-half width for the SO z tiles

    nc = bacc.Bacc(
        "TRN2", target_bir_lowering=False, debug=False, num_devices=num_devices
    )

    def din(name, shape, dt=F32):
        return nc.dram_tensor(name, shape, dt, kind="ExternalInput").ap()

    eng_ks = _cmp_engines(nkc)
    islt_ks = _islt_ks(nkc)
    n_islt = len(islt_ks)

    # packed small constants: one bf16 blob + one f32 blob, sliced on SBUF
    ob_blhs, ob_xc2d = 0, 3 * nkc
    ob_i128 = ob_xc2d + 2 * n_islt
    ob_onesc = ob_i128 + P
    wb = ob_onesc + 1
    NB = 64                         # rank buckets for the M' estimate
    of_xcol, of_xhc = 0, nkc
    of_lob = of_xhc + nihc
    of_hib = of_lob + NB
    of_i6 = of_hib + NB
    wf = of_i6 + 6

    xbf = din("xbf", [P, nih], F32)        # x of own i-half, broadcast 128x
    l9full = din("l9full", [9, nj], BF16)  # host-stacked z lhs rows
    pkf = din("pkf", [P, wf], F32)
    pkb = din("pkb", [P, wb], BF16)
    xr6 = din("xr6", [6, nih], BF16)       # r9 rows 0-5, own half
    xr6o = din("xr6o", [6, nih], BF16)     # r9 rows 0-5, partner half
    xballh = din("xballh", [P, nihc, 5], BF16)  # own cols [xch,xcl,0,0,1]

    # output in [j, i] layout, bf16; host transposes while unsharding
    out = nc.dram_tensor("out", [nj, n], BF16, kind="ExternalOutput").ap()

    # exchange payload (bf16): the -B splits in (s, t, p) order
    nsp = 3 * nihc                  # -B split columns per partition
    npay = P * nsp
    bh_dram = nc.dram_tensor("bh_dram", [1, npay], BF16).ap()
    nhalves = 1 if single else 2
    bfull_dram = nc.dram_tensor("bfull_dram", [nhalves, npay], BF16).ap()
    groups = [[2 * p, 2 * p + 1] for p in range(max(1, num_devices // 2))]

    def mm512(out_ap, lhsT, rhs, start=True, stop=True):
        """matmul with the moving dim split into <=512-column chunks."""
        nfree = rhs.shape[-1]
        assert out_ap.shape[-1] == nfree
        for o in range(0, nfree, 512):
            e = min(o + 512, nfree)
            nc.tensor.matmul(
                out_ap[..., o:e], lhsT, rhs[..., o:e], start=start, stop=stop
            )

    with tile.TileContext(nc) as tc, ExitStack() as ctx:
        cpool = ctx.enter_context(tc.tile_pool(name="consts", bufs=1))

        def load(pool, ap_dram, shape, dt, name):
            t = pool.tile(shape, dt, tag=name)
            nc.sync.dma_start(out=t[:], in_=ap_dram)
            return t

        # input loads, critical-path first (xb halved so chunk-0 comparisons
        # can start on the first half)
        xb = cpool.tile([P, nih], F32, tag="xb")
        nc.sync.dma_start(out=xb[:, 0 : nih // 2], in_=xbf[:, 0 : nih // 2])
        pkf_s = load(cpool, pkf, [P, wf], F32, "pkf")
        nc.sync.dma_start(out=xb[:, nih // 2 : nih], in_=xbf[:, nih // 2 : nih])
        pkb_s = load(cpool, pkb, [P, wb], BF16, "pkb")
        l9 = load(cpool, l9full, [9, nj], BF16, "l9")
        r9 = [
            cpool.tile([9, nih], BF16, tag=f"r9_{h}", name=f"r9_{h}")
            for h in range(nhalves)
        ]
        nc.sync.dma_start(out=r9[0][3:9, :], in_=xr6)
        if nhalves == 2:
            nc.sync.dma_start(out=r9[1][3:9, :], in_=xr6o)
        rep9 = cpool.tile([9, NB], BF16, tag="rep9")
        nmcol = cpool.tile([P, n // P if single else n // 2 // P], F32,
                           tag="nmcol")

        xcol_s = pkf_s[:, of_xcol : of_xcol + nkc]
        xhc_s = pkf_s[:, of_xhc : of_xhc + nihc]
        lob_s = pkf_s[:, of_lob : of_lob + NB]
        hib_s = pkf_s[:, of_hib : of_hib + NB]
        i6f_s = pkf_s[0:6, of_i6 : of_i6 + 6]
        blhs_s = pkb_s[:, ob_blhs : ob_blhs + 3 * nkc]
        xc2d_s = pkb_s[:, ob_xc2d : ob_xc2d + 2 * n_islt]
        i128_s = pkb_s[:, ob_i128 : ob_i128 + P]
        onesc_s = pkb_s[:, ob_onesc : ob_onesc + 1]

        # ---- PE warm-up: keep the p-state ramp going while inputs load,
        # and preload the ACT function table with a dummy Sign ----
        # SO-loop SBUF pools created BEFORE the prep pool so their tiles
        # never alias prep scratch (aliasing would chain the first exp
        # behind the last prep op through an SBUF reuse WAR)
        dpool = ctx.enter_context(tc.tile_pool(name="dd", bufs=6))
        outp = ctx.enter_context(tc.tile_pool(name="outp", bufs=5))
        wt = cpool.tile([P, 512], BF16, tag="wt")
        with (
            tc.tile_pool(name="warmp", bufs=1, space="PSUM") as wpp,
        ):
            nc.vector.memset(wt[:], 1.0)
            wsg = cpool.tile([1, 1], BF16, tag="wsg")
            nc.scalar.activation(out=wsg[:], in_=wt[0:1, 0:1], func=AF.Sign)
            wps = wpp.tile([P, 512], F32)
            for _ in range(10):
                nc.tensor.matmul(
                    wps[:], wt[:, 0:P], wt[:], start=True, stop=True
                )

        with tc.tile_pool(name="prep", bufs=1) as pp_s:
            # comparison pass across DVE/Pool (is_lt -> {0,1}) and ACT
            # (Sign -> {-1,0,1}); PE drains g tiles in program order. For
            # the is_lt set, sum_k |x_i-x_k| = x_i*(nD - 2*cntD) - SD +
            # 2*tD; for the ACT set it's x_i*sgnS - tS. Ties vanish either
            # way.
            nxcol = pp_s.tile([P, nkc], F32, tag="nxcol")
            nc.vector.tensor_scalar_mul(nxcol[:], xcol_s, -1.0)
            b3 = pp_s.tile([3, nih], F32, tag="b3")
            b3s = pp_s.tile([3, nih], F32, tag="b3s")
            with (
                tc.tile_pool(name="bp", bufs=1, space="PSUM") as bp,
                tc.tile_pool(name="gp", bufs=6) as gp,
            ):
                bpsum = bp.tile([3, nih], F32)
                bpsum2 = bp.tile([3, nih], F32)
                nlt_seen = nact_seen = 0
                for k in range(nkc):
                    g = gp.tile([P, nih], BF16, tag="g")
                    e = eng_ks[k]
                    if e != "a":
                        nlt_seen += 1
                        veng = nc.vector if e == "d" else nc.gpsimd
                        spans = (
                            [(0, nih // 2), (nih // 2, nih)]
                            if nlt_seen <= 2
                            else [(0, nih)]
                        )
                        for a0, a1 in spans:
                            veng.tensor_scalar(
                                out=g[:, a0:a1],
                                in0=xb[:, a0:a1],
                                scalar1=xcol_s[:, k : k + 1],
                                scalar2=None,
                                op0=ALU.is_lt,
                            )
                        mm512(
                            bpsum[:],
                            blhs_s[:, 3 * k : 3 * k + 3],
                            g[:],
                            start=(nlt_seen == 1),
                            stop=(nlt_seen == n_islt),
                        )
                    else:
                        nact_seen += 1
                        nc.scalar.activation(
                            out=g[:],
                            in_=xb[:],
                            func=AF.Sign,
                            bias=nxcol[0:P, k : k + 1],
                        )
                        mm512(
                            bpsum2[:],
                            blhs_s[:, 3 * k : 3 * k + 3],
                            g[:],
                            start=(nact_seen == 1),
                            stop=(nact_seen == nkc - n_islt),
                        )
                # drain both accumulators, split across DVE and ACT so the
                # copies overlap (compute APs must start at partition 0)
                hw2 = nih // 2
                nc.vector.tensor_copy(out=b3[:, 0:hw2], in_=bpsum[:, 0:hw2])
                nc.scalar.activation(
                    out=b3[:, hw2:nih], in_=bpsum[:, hw2:nih], func=AF.Copy
                )
                nc.vector.tensor_copy(out=b3s[:, 0:hw2], in_=bpsum2[:, 0:hw2])
                nc.scalar.activation(
                    out=b3s[:, hw2:nih], in_=bpsum2[:, hw2:nih], func=AF.Copy
                )

            # rows -> columns via tiny PE transposes (exact data movement)
            bc_all = pp_s.tile([P, nihc, 6], F32, tag="bc_all")
            with tc.tile_pool(name="tp", bufs=1, space="PSUM") as tp:
                bc_ps = tp.tile([P, nihc, 6], F32)
                for ch in range(nihc):
                    nc.tensor.transpose(
                        bc_ps[:, ch, 0:3],
                        b3[:, ch * P : (ch + 1) * P],
                        i6f_s[0:3, 0:3],
                    )
                    nc.tensor.transpose(
                        bc_ps[:, ch, 3:6],
                        b3s[:, ch * P : (ch + 1) * P],
                        i6f_s[0:3, 0:3],
                    )
                nc.vector.tensor_copy(out=bc_all[:], in_=bc_ps[:])
            cntc = bc_all[:, :, 0]
            thc = bc_all[:, :, 1]
            tlc = bc_all[:, :, 2]
            sgnc = bc_all[:, :, 3]
            tshc = bc_all[:, :, 4]
            tslc = bc_all[:, :, 5]

            # +SD (sum of x over the is_lt-chunk k's)
            spos = pp_s.tile([1, 1], F32, tag="spos")
            with tc.tile_pool(name="sp", bufs=1, space="PSUM") as sp:
                sxp = sp.tile([1, 2 * n_islt], F32)
                nc.tensor.matmul(sxp[:], onesc_s, xc2d_s, start=True, stop=True)
                nc.vector.tensor_reduce(
                    out=spos[:], in_=sxp[:], axis=mybir.AxisListType.X, op=ALU.add
                )
            sposc = pp_s.tile([P, 1], F32, tag="sposc")
            nc.gpsimd.partition_broadcast(sposc[:], spos[0:1, 0:1])

            # -B = -x*(nD - 2*cntD + sgnS) + SD - 2*(tDh+tDl) + (tSh+tSl)
            nD = float(n_islt * P)
            r1 = pp_s.tile([P, nihc], F32, tag="r1")
            nc.vector.tensor_scalar(
                out=r1[:],
                in0=cntc,
                scalar1=-2.0,
                scalar2=nD,
                op0=ALU.mult,
                op1=ALU.add,
            )
            r1b = pp_s.tile([P, nihc], F32, tag="r1b")
            nc.vector.tensor_tensor(out=r1b[:], in0=r1[:], in1=sgnc, op=ALU.add)
            r2n = pp_s.tile([P, nihc], F32, tag="r2n")
            nc.vector.scalar_tensor_tensor(
                out=r2n[:], in0=xhc_s, scalar=-1.0, in1=r1b[:],
                op0=ALU.mult, op1=ALU.mult,
            )
            tt = pp_s.tile([P, nihc], F32, tag="tt")
            nc.vector.tensor_tensor(out=tt[:], in0=thc, in1=tlc, op=ALU.add)
            u1 = pp_s.tile([P, nihc], F32, tag="u1")
            nc.vector.scalar_tensor_tensor(
                out=u1[:], in0=tt[:], scalar=-2.0, in1=r2n[:],
                op0=ALU.mult, op1=ALU.add,
            )
            tts = pp_s.tile([P, nihc], F32, tag="tts")
            nc.vector.tensor_tensor(out=tts[:], in0=tshc, in1=tslc, op=ALU.add)
            u2 = pp_s.tile([P, nihc], F32, tag="u2")
            nc.vector.tensor_tensor(out=u2[:], in0=u1[:], in1=tts[:], op=ALU.add)
            nbhalfc = pp_s.tile([P, nihc], F32, tag="nbhalfc")
            nc.vector.tensor_scalar(
                out=nbhalfc[:],
                in0=u2[:],
                scalar1=sposc[:, 0:1],
                scalar2=None,
                op0=ALU.add,
            )

            # -B bf16 splits, chunk-major [P, t, s] so each chunk's three
            # split columns sit adjacent for the PE row-transposes below
            nbsh = pp_s.tile([P, nihc, 3], BF16, tag="nbsh")
            sp0 = nbsh[:, :, 0]
            sp1 = nbsh[:, :, 1]
            sp2 = nbsh[:, :, 2]
            cs_t1 = pp_s.tile([P, nihc], F32, tag="cs_t1")
            cs_t2 = pp_s.tile([P, nihc], F32, tag="cs_t2")
            nc.vector.tensor_copy(out=sp0, in_=nbhalfc[:])
            nc.vector.tensor_tensor(out=cs_t1[:], in0=nbhalfc[:], in1=sp0,
                                    op=ALU.subtract)
            nc.vector.tensor_copy(out=sp1, in_=cs_t1[:])
            nc.vector.tensor_tensor(out=cs_t2[:], in0=cs_t1[:], in1=sp1,
                                    op=ALU.subtract)
            nc.vector.tensor_copy(out=sp2, in_=cs_t2[:])

            # ---- -B rows straight into r9[0] via tiny PE transposes (PE
            # and ACT are idle here; skips the slow element-scatter DMA),
            # then one contiguous DMA publishes them as the payload ----
            with tc.tile_pool(name="btp", bufs=1, space="PSUM") as btp:
                btr = btp.tile([3, nih], BF16)
                for ch in range(nihc):
                    nc.tensor.transpose(
                        btr[:, ch * P : (ch + 1) * P], nbsh[:, ch, :], i128_s
                    )
                nc.scalar.activation(out=r9[0][0:3, :], in_=btr[:], func=AF.Copy)
            nc.sync.dma_start(
                out=bh_dram[0, 0 : P * nsp].rearrange("(s i) -> s i", s=3),
                in_=r9[0][0:3, :],
            )
            if use_collective:
                nc.gpsimd.collective_compute(
                    "AllGather",
                    ALU.bypass,
                    replica_groups=groups,
                    ins=[bh_dram],
                    outs=[bfull_dram],
                )
            else:
                for hh in range(nhalves):
                    nc.sync.dma_start(out=bfull_dram[hh : hh + 1, :], in_=bh_dram)

            if nhalves == 2:
                # AllGather slots are by group position, so slot 1 is this
                # very core on odd ranks; the payload slabs are read here
                # and the position-free recovery runs AFTER the M' chain so
                # the collective never blocks the own-half exp stream.
                s01 = pp_s.tile([3, 2, nih], BF16, tag="s01")
                nc.sync.dma_start(
                    out=s01[:],
                    in_=bfull_dram[0:2, 0 : P * nsp].rearrange(
                        "h (s i) -> s h i", s=3
                    ),
                )

            # ---- own-half rank-bucket partial sums (pre-exchange) ----
            xballh_s = load(pp_s, xballh, [P, nihc, 5], BF16, "xballh")
            nc.vector.tensor_copy(out=xballh_s[:, :, 2], in_=nbsh[:, :, 0])
            nc.vector.tensor_copy(out=xballh_s[:, :, 3], in_=nbsh[:, :, 1])
            reps_own = pp_s.tile([NB, 5], F32, tag="reps_own")
            with (
                tc.tile_pool(name="repp0", bufs=1, space="PSUM") as repp0,
                tc.tile_pool(name="mkp0", bufs=8) as mkp0,
            ):
                repso_p = repp0.tile([NB, 5], F32)
                nrch = max(1, (3 * nihc) // 4)  # a subset stays a valid
                for ch in range(nrch):          # max-underestimate (convexity)
                    m1 = mkp0.tile([P, NB], BF16, tag="m1")
                    nc.vector.tensor_scalar(
                        out=m1[:],
                        in0=lob_s,
                        scalar1=r1b[:, ch : ch + 1],
                        scalar2=None,
                        op0=ALU.is_le,
                    )
                    msk = mkp0.tile([P, NB], BF16, tag="msk")
                    nc.vector.scalar_tensor_tensor(
                        out=msk[:],
                        in0=hib_s,
                        scalar=r1b[:, ch : ch + 1],
                        in1=m1[:],
                        op0=ALU.is_gt,
                        op1=ALU.mult,
                    )
                    nc.tensor.matmul(
                        repso_p[:],
                        msk[:],
                        xballh_s[:, ch, :],
                        start=(ch == 0),
                        stop=(ch == nrch - 1),
                    )
                nc.vector.tensor_copy(out=reps_own[:], in_=repso_p[:])
            # ---- rank-bucket representatives -> rep9 stack ----
            # OWN-half reps only: the top ranks of every column are never
            # all in the partner half (P ~ 2^-30), so the own-half bucket
            # max underestimates each column max by only a few more units
            # than the full-data version -- still far inside the exp(88)
            # bf16 budget, and softmax shift-invariance keeps the result
            # exact. This takes the whole M' pipeline off the exchange
            # critical path.
            # reps rows: [sum xh, sum xl, sum -Bh, sum -Bm, count]
            reps = reps_own
            cnt1 = pp_s.tile([NB, 1], F32, tag="cnt1")
            nc.vector.tensor_scalar_max(cnt1[:], reps[:, 4:5], 1.0)
            rc = pp_s.tile([NB, 1], F32, tag="rc")
            nc.vector.reciprocal(rc[:], cnt1[:])
            repx = pp_s.tile([NB, 1], F32, tag="repx")
            nc.vector.tensor_tensor(
                out=repx[:], in0=reps[:, 0:1], in1=reps[:, 1:2], op=ALU.add
            )
            nc.vector.tensor_tensor(
                out=repx[:], in0=repx[:], in1=rc[:], op=ALU.mult
            )
            repb = pp_s.tile([NB, 1], F32, tag="repb")  # mean of -B
            nc.vector.tensor_tensor(
                out=repb[:], in0=reps[:, 2:3], in1=reps[:, 3:4], op=ALU.add
            )
            nc.vector.tensor_tensor(
                out=repb[:], in0=repb[:], in1=rc[:], op=ALU.mult
            )
            # empty bucket -> push its line to -inf
            iz = pp_s.tile([NB, 1], F32, tag="iz")
            nc.vector.tensor_scalar(
                out=iz[:], in0=reps[:, 4:5], scalar1=0.5, scalar2=None,
                op0=ALU.is_le,
            )
            nc.vector.scalar_tensor_tensor(
                out=repb[:], in0=iz[:], scalar=-1e30, in1=repb[:],
                op0=ALU.mult, op1=ALU.add,
            )
            # rep9 columns pre-transpose: [nBh2,nBm2,0,xh2,xh2,xl2,xl2,0,0]
            rs9 = pp_s.tile([NB, 9], BF16, tag="rs9")
            rtmp = pp_s.tile([NB, 1], F32, tag="rep_rt")
            nc.vector.tensor_copy(out=rs9[:, 0:1], in_=repb[:])
            nc.vector.tensor_tensor(
                out=rtmp[:], in0=repb[:], in1=rs9[:, 0:1], op=ALU.subtract
            )
            nc.vector.tensor_copy(out=rs9[:, 1:2], in_=rtmp[:])
            nc.vector.memset(rs9[:, 2:3], 0.0)
            nc.vector.tensor_copy(out=rs9[:, 3:4], in_=repx[:])
            nc.vector.tensor_copy(out=rs9[:, 4:5], in_=rs9[:, 3:4])
            nc.vector.tensor_tensor(
                out=rtmp[:], in0=repx[:], in1=rs9[:, 3:4], op=ALU.subtract
            )
            nc.vector.tensor_copy(out=rs9[:, 5:6], in_=rtmp[:])
            nc.vector.tensor_copy(out=rs9[:, 6:7], in_=rs9[:, 5:6])
            nc.vector.memset(rs9[:, 7:9], 0.0)
            with tc.tile_pool(name="repp", bufs=1, space="PSUM") as repp:
                p9r = repp.tile([9, NB], F32)
                nc.tensor.matmul(
                    p9r[:], rs9[:], i128_s[0:NB, 0:NB], start=True, stop=True
                )
                nc.vector.tensor_copy(out=rep9[:], in_=p9r[:])

            # ---- M' for every j-chunk upfront (needs only rep9 + l9, so
            # this overlaps the collective): z at the 128 bucket reps per
            # chunk, DVE max-reduces, negate. The placeholder pool pins
            # zrep to the upper PSUM banks so the SO loop's first z tile
            # (lower banks) doesn't wait for the M' reduces. ----
            with tc.tile_pool(name="mrp", bufs=1, space="PSUM") as mrp:
                zrep = mrp.tile([P, njc, NB], F32)
                for jc in range(njc):
                    nc.tensor.matmul(
                        zrep[:, jc, :],
                        l9[:, jc * P : (jc + 1) * P],
                        rep9[:],
                        start=True,
                        stop=True,
                    )
                mcol = pp_s.tile([P, njc], F32, tag="mcol")
                nsp0 = min(4, njc)
                nc.vector.tensor_reduce(
                    out=mcol[:, 0:nsp0], in_=zrep[:, 0:nsp0, :],
                    axis=mybir.AxisListType.X, op=ALU.max,
                )
                nc.vector.tensor_scalar_mul(
                    nmcol[:, 0:nsp0], mcol[:, 0:nsp0], -1.0
                )
                if nhalves == 2:
                    # partner -B rows = (slab0 + slab1) - own: exact, since
                    # within-row magnitudes are homogeneous bf16 values so
                    # the f32 sums round-trip exactly. Ordered between the
                    # M' reduces: the first chunks' bias is urgent, the
                    # rest isn't, and q1 fills need these rows soon.
                    ssum2 = pp_s.tile([3, nih], F32, tag="ssum2")
                    nc.vector.tensor_tensor(
                        out=ssum2[:], in0=s01[:, 0, :], in1=s01[:, 1, :],
                        op=ALU.add,
                    )
                    nc.vector.tensor_tensor(
                        out=r9[1][0:3, :], in0=ssum2[:], in1=r9[0][0:3, :],
                        op=ALU.subtract,
                    )
                if njc > nsp0:
                    nc.vector.tensor_reduce(
                        out=mcol[:, nsp0:njc], in_=zrep[:, nsp0:njc, :],
                        axis=mybir.AxisListType.X, op=ALU.max,
                    )
                    nc.vector.tensor_scalar_mul(
                        nmcol[:, nsp0:njc], mcol[:, nsp0:njc], -1.0
                    )

        # ---------------- Phase SO: merged softmax+output per j-chunk -------
        # software-pipelined half-streams: the own-half (q=0) exp of chunk c
        # is issued before the partner-half (q=1) exp of chunk c-1, so the
        # ACT stream starts as soon as r9[0] lands -- before the collective
        # delivers r9[1].
        spool = ctx.enter_context(tc.tile_pool(name="sz", bufs=2, space="PSUM"))
        nhi = n // ih  # i-halves per chunk

        def z_half(zp, lhs, q):
            h, qq = divmod(q * ih, nih)
            o = 0
            while o < ih:
                hh, qo = h, qq + o
                if qo >= nih:
                    hh, qo = h + 1, qo - nih
                e = min(qo + 512, nih) - qo
                nc.tensor.matmul(
                    zp[:, o : o + e],
                    lhs,
                    r9[hh][:, qo : qo + e],
                    start=True,
                    stop=True,
                )
                o += e

        def finalize(st):
            ot, dq = st["ot"], st["dq"]
            dsum = dpool.tile([P, 1], F32, tag="dsum")
            nc.vector.tensor_tensor(
                out=dsum[:], in0=dq[:, 0:1], in1=dq[:, 1:2], op=ALU.add
            )
            rcp = dpool.tile([P, 1], F32, tag="rcp")
            nc.vector.reciprocal(rcp[:], dsum[:])
            npc = 4 if st["jc"] == njc - 1 else 2
            for hh in range(npc):
                sl = slice(hh * (n // npc), (hh + 1) * (n // npc))
                nc.vector.tensor_scalar(
                    out=ot[:, sl],
                    in0=ot[:, sl],
                    scalar1=rcp[:, 0:1],
                    scalar2=None,
                    op0=ALU.mult,
                )
                nc.sync.dma_start(
                    out=out.rearrange("(jc p) i -> p jc i", p=P)[
                        :, st["jc"], sl
                    ],
                    in_=ot[:, sl],
                )

        skew = spool.tile([P, ih], F32, tag="sz", name="skew")  # rotation
        # skew: the first real z tile must not alias the M' zrep banks
        DEPTH = 2  # own-half stream runs this many chunks ahead of the
        pending = []  # partner-half stream (collective latency headroom)
        for jc in range(njc + DEPTH):
            if jc < njc:
                lhs = l9[:, jc * P : (jc + 1) * P]
                st = {
                    "jc": jc,
                    "ot": outp.tile([P, n], BF16, tag="ot", name="ot"),
                    "dq": dpool.tile([P, 2], F32, tag="dq", name="dq"),
                    "lhs": lhs,
                }
                zp = spool.tile([P, ih], F32, tag="sz")
                z_half(zp, lhs, 0)
                nc.scalar.activation(
                    out=st["ot"][:, 0:ih],
                    in_=zp[:],
                    func=AF.Exp,
                    bias=nmcol[0:P, jc : jc + 1],
                    scale=1.0,
                    accum_out=st["dq"][:, 0:1],
                )
                pending.append(st)
            if len(pending) > DEPTH or jc >= njc:
                prev = pending.pop(0)
                zp1 = spool.tile([P, ih], F32, tag="sz")
                z_half(zp1, prev["lhs"], 1)
                nc.scalar.activation(
                    out=prev["ot"][:, ih : 2 * ih],
                    in_=zp1[:],
                    func=AF.Exp,
                    bias=nmcol[0:P, prev["jc"] : prev["jc"] + 1],
                    scale=1.0,
                    accum_out=prev["dq"][:, 1:2],
                )
                finalize(prev)

    nc.compile()
    return nc


# ---------------------------------------------------------------------------


def make_in_maps(scores, n, mode="pair"):
    """Per-core input dicts. Core c -> batch c//2, halves h = c%2."""
    single = mode == "single"
    nj = n if single else n // 2
    nih = n if single else n // 2
    nkc = n // P
    nihc = nih // P
    ncores = 1 if single else N_CORES

    cfull = (2 * np.arange(n) + 1 - n).astype(np.float32)
    ch_f, cl_f = _split2(cfull)

    islt_ks = _islt_ks(nkc)
    n_islt = len(islt_ks)

    in_maps = []
    for c in range(ncores):
        b = 0 if single else c // 2
        h = 0 if single else c % 2
        x = np.asarray(scores[b], dtype=np.float32)
        xh_, xm_, xl_ = _split3(x)
        xch, xcl = _split2(x)
        xcol = np.ascontiguousarray(x.reshape(nkc, P).T).astype(np.float32)
        xchc = np.ascontiguousarray(xch.reshape(nkc, P).T)
        xclc = np.ascontiguousarray(xcl.reshape(nkc, P).T)
        blhs = np.zeros((P, 3 * nkc), dtype=ml_dtypes.bfloat16)
        blhs[:, 0::3] = 1.0
        blhs[:, 1::3] = xchc
        blhs[:, 2::3] = xclc
        xc2d = np.concatenate([xchc[:, islt_ks], xclc[:, islt_ks]], axis=1)
        assert xc2d.shape[1] == 2 * n_islt
        sl = slice(h * nih, h * nih + nih)
        sj = slice(h * nj, h * nj + nj)
        so = slice((1 - h) * nih, (1 - h) * nih + nih) if not single else sl

        def xr6_of(s):
            return np.stack(
                [xh_[s], xh_[s], xm_[s], xm_[s], xl_[s], xl_[s]], axis=0
            )

        ones_j = np.ones((3, nj), dtype=ml_dtypes.bfloat16)
        l9full = np.concatenate(
            [
                ones_j,
                ch_f[None, sj], cl_f[None, sj],
                ch_f[None, sj], cl_f[None, sj],
                ch_f[None, sj], cl_f[None, sj],
            ],
            axis=0,
        )
        NB = 64
        lo_row = (-n + np.arange(NB) * (2 * n // NB)).astype(np.float32)
        lob = np.tile(lo_row[None, :], (P, 1))
        hib = lob + float(2 * n // NB)
        xballh = np.zeros((P, nihc, 5), dtype=ml_dtypes.bfloat16)
        xballh[:, :, 0] = xchc[:, h * nihc : (h + 1) * nihc]
        xballh[:, :, 1] = xclc[:, h * nihc : (h + 1) * nihc]
        xballh[:, :, 4] = 1.0

        wb = 3 * nkc + 2 * n_islt + P + 1
        pkb = np.zeros((P, wb), dtype=ml_dtypes.bfloat16)
        o = 0
        pkb[:, o : o + 3 * nkc] = blhs
        o += 3 * nkc
        pkb[:, o : o + 2 * n_islt] = xc2d
        o += 2 * n_islt
        pkb[:, o : o + P] = np.eye(P, dtype=ml_dtypes.bfloat16)
        o += P
        pkb[:, o] = 1.0  # onesc
        o += 1
        assert o == wb

        wf = nkc + nihc + NB + NB + 6
        pkf = np.zeros((P, wf), dtype=np.float32)
        o = 0
        pkf[:, o : o + nkc] = xcol
        o += nkc
        pkf[:, o : o + nihc] = np.ascontiguousarray(x[sl].reshape(-1, P).T)
        o += nihc
        pkf[:, o : o + NB] = lob
        o += NB
        pkf[:, o : o + NB] = hib
        o += NB
        pkf[0:6, o : o + 6] = np.eye(6, dtype=np.float32)
        o += 6
        assert o == wf

        in_maps.append(
            {
                "xbf": np.tile(x[sl][None, :], (P, 1)),
                "l9full": l9full,
                "pkf": pkf,
                "pkb": pkb,
                "xr6": xr6_of(sl),
                "xr6o": xr6_of(so),
                "xballh": xballh,
            }
        )
    return in_maps


_NC_CACHE = {}


def _get_nc(n):
    if n not in _NC_CACHE:
        _NC_CACHE[n] = build_nc(n=n, mode="pair", num_devices=N_CORES)
    return _NC_CACHE[n]


def kernel(scores):
    scores = np.asarray(scores, dtype=np.float32)
    b, n = scores.shape
    nj = n // 2
    nih = n // 2
    nc = _get_nc(n)
    in_maps = make_in_maps(scores, n, mode="pair")
    res = run_bass_kernel_spmd(nc, in_maps, list(range(N_CORES)))
    out = np.empty((b, n, n), dtype=np.float32)
    for c in range(N_CORES):
        bb, h = c // 2, c % 2
        odev = np.asarray(res.results[c]["out"], dtype=np.float32)  # [nj, n]
        # odev columns: [own half (i in h-half) | partner half]
        out[bb, h * nih : (h + 1) * nih, h * nj : (h + 1) * nj] = odev[
            :, 0:nih
        ].T
        out[bb, (1 - h) * nih : (2 - h) * nih, h * nj : (h + 1) * nj] = odev[
            :, nih : 2 * nih
        ].T
    return out


# revision 24
# speedup vs baseline: 2.3025x; 1.0161x over previous
"""NeuralSort relaxed-permutation kernel for 8 Trainium2 NeuronCores.

out[b, i, j] = softmax_i( s_i * scaling_j - B_i ),  s = -scores[b]
  scaling_j = n - 1 - 2j   =>  z[j,i] = c_j * x_i - B_i  with x = scores[b],
  c_j = -(n - 1 - 2j) = 2j + 1 - n
  B_i = sum_k |x_i - x_k| = x_i*(n - 2*cnt_i) - S + 2*t_i
        cnt_i = #{k: x_k > x_i},  t_i = sum_{k: x_k > x_i} x_k,  S = sum_k x_k

Sharding: core c -> (batch b = c//2, j-half h = c%2). Each core emits the
half-j (n/2) by full-i (n) slab of batch b in [j, i] layout (bf16); the host
transposes while unsharding.

Per-core pipeline (all z matmuls bf16 @ 1 cyc/row, exact via hi/mid/lo
splits):
  P: comparison tiles split across DVE/Pool (is_lt -> {0,1}) and ACT (Sign ->
     {-1,0,1}); PE reduces them with [ones|x_hi|x_lo] stationaries into
     cnt/t rows (PE is primed with dummy matmuls during input load so the
     reduction runs at full p-state). Row results go to column layout via
     tiny PE transposes; combine to -B columns; r1b = signed rank.
  X: AllGather one bf16 payload within the batch pair: [-B bf16 splits
     (column dump) || rank-bucket partial sums as hi/lo bf16 pairs]. The
     -B split rows for the z stacks are scatter-read straight out of the
     payload; x/c rows of the stacks come pre-stacked from the host.
  M: i's are bucketed into 128 rank ranges by r1b (mask tiles on DVE+Pool,
     PE mask-matmul -> per-bucket mean (x_bar, -B_bar)). z at the bucket
     means underestimates each column max by <~40 (z is concave in rank
     space; B_bar >= f(x_bar) by convexity), all the exp shift needs.
  SO (merged stats+output): per 128-j chunk, i on the free axis: rep-z
     matmul + DVE rowmax -> M'_jc just-in-time; z via K=9 matmul into PSUM
     (two 2048-i halves, double buffered); ONE ACT exp(z - M') -> bf16 out
     tile; DVE half-reduces -> D, reciprocal, in-place rescale by 1/D; one
     1 MiB DMA per chunk with 8 KiB contiguous rows. Softmax is
     shift-invariant so exp(z - M')/D is exact regardless of M' slack.
"""

from contextlib import ExitStack

import numpy as np
import ml_dtypes

import concourse.bass as bass
import concourse.tile as tile
from concourse import bacc, mybir
from concourse.bass_utils import run_bass_kernel_spmd

F32 = mybir.dt.float32
BF16 = mybir.dt.bfloat16
AF = mybir.ActivationFunctionType
ALU = mybir.AluOpType

N_CORES = 8
P = 128

# z = sum_k l9[k] * r9[k]; rows ordered so the device-computed -B rows sit
# at partition base 0 (matmul lhsT slices need base 0/32/64) and the
# host-fed x rows are 3-8:
#   l9 = [ 1,   1,   1,  chi, clo, chi, clo, chi, clo]
#   r9 = [-Bh, -Bm, -Bl, xh,  xh,  xm,  xm,  xl,  xl]


def _bf(x):
    return np.asarray(x, dtype=ml_dtypes.bfloat16)


def _split3(x):
    x = np.asarray(x, dtype=np.float32)
    h = _bf(x)
    r = x - h.astype(np.float32)
    m = _bf(r)
    l = _bf(r - m.astype(np.float32))
    return h, m, l


def _split8(x, terms=6):
    """x ~= sum_j s_j * 16**-j with s_j fp8 e4m3; residual ~|x| 2^-24."""
    x = np.asarray(x, dtype=np.float32)
    out = []
    r = x.copy()
    for _ in range(terms):
        q = np.asarray(r, dtype=ml_dtypes.float8_e4m3fn)
        out.append(q)
        r = (r - q.astype(np.float32)) * 16.0
    return out


def _split2(x):
    x = np.asarray(x, dtype=np.float32)
    h = _bf(x)
    l = _bf(x - h.astype(np.float32))
    return h, l


def _cmp_engines(nkc):
    """Comparison-chunk engine assignment: 'd' DVE is_lt, 'a' ACT Sign.
    (Pool cannot run TensorScalarPtr on TRN2.) Rates ~1127/1892 ns per
    chunk -> 5:3 mix keeps both generators finishing together while PE
    (852 ns/chunk ramped) stays the binding resource."""
    pat = ["d", "a", "d", "d", "a", "d", "a", "d"]
    eng = [pat[k % len(pat)] for k in range(nkc)]
    if nkc >= 2 and eng[-1] == "a":
        eng[-1], eng[-2] = eng[-2], eng[-1]
    return eng


def _islt_ks(nkc):
    """Chunks whose comparisons use is_lt (DVE+Pool); rest use ACT Sign."""
    eng = _cmp_engines(nkc)
    return [k for k in range(nkc) if eng[k] != "a"]


def build_nc(n=4096, mode="pair", num_devices=N_CORES):
    """mode: "pair" (8-core, AllGather within batch pairs); "single" (1-core
    debug: full j/i ranges, no collective); "timing" (pair shapes,
    collective replaced by local copies -- for the 1-core timeline model)."""
    single = mode == "single"
    use_collective = mode == "pair"
    nj = n if single else n // 2    # output columns (j) per core
    nih = n if single else n // 2   # i-range whose B this core computes
    nkc = n // P                    # k-chunks in the comparison pass
    njc = nj // P                   # 128-wide j-chunks
    nihc = nih // P                 # 128-wide i-chunks of the own half
    ih = n // 2                     # i-half width for the SO z tiles

    nc = bacc.Bacc(
        "TRN2", target_bir_lowering=False, debug=False, num_devices=num_devices
    )

    def din(name, shape, dt=F32):
        return nc.dram_tensor(name, shape, dt, kind="ExternalInput").ap()

    eng_ks = _cmp_engines(nkc)
    islt_ks = _islt_ks(nkc)
    n_islt = len(islt_ks)
    sign_ks = [k for k in range(nkc) if k not in islt_ks]
    # same-type chunk pairs for the fp8 DoubleRow reduction
    prs = [(islt_ks[i], islt_ks[i + 1]) for i in range(0, n_islt, 2)] + [
        (sign_ks[i], sign_ks[i + 1]) for i in range(0, len(sign_ks), 2)
    ]
    # interleave islt/sign pairs ~5:3 so both generator engines stay busy
    def _pair_order():
        di = [p for p in prs if p[0] in islt_ks]
        ai = [p for p in prs if p[0] in sign_ks]
        pat = ["d", "a", "d", "d", "a", "d", "a", "d"]
        seq = []
        while di or ai:
            for c in pat:
                if c == "d" and di:
                    seq.append(("d", di.pop(0)))
                elif c == "a" and ai:
                    seq.append(("a", ai.pop(0)))
        return seq
    pair_seq = _pair_order()
    KR = 7   # data rows per chunk: [1, s0..s5]
    KRP = 16  # padded stride: DoubleRow needs dim-1 stride % 16 bytes == 0

    # packed small constants: one bf16 blob + one f32 blob, sliced on SBUF
    ob_xc2d = 0
    ob_i128 = ob_xc2d + 2 * n_islt
    ob_onesc = ob_i128 + P
    wb = ob_onesc + 1
    NB = 64                         # rank buckets for the M' estimate
    of_xcol, of_xhc = 0, nkc
    of_lob = of_xhc + nihc
    of_hib = of_lob + NB
    of_i6 = of_hib + NB
    wf = of_i6 + 7

    xbf = din("xbf", [P, nih], F32)        # x of own i-half, broadcast 128x
    l9full = din("l9full", [9, nj], BF16)  # host-stacked z lhs rows
    pkf = din("pkf", [P, wf], F32)
    pkb = din("pkb", [P, wb], BF16)
    xr6 = din("xr6", [6, nih], BF16)       # r9 rows 0-5, own half
    xr6o = din("xr6o", [6, nih], BF16)     # r9 rows 0-5, partner half
    xballh = din("xballh", [P, nihc, 5], BF16)  # own cols [xch,xcl,0,0,1]
    # fp8 comparison-reduction stationaries, pair-ordered: per chunk 7 cols
    # [1, s0..s5] where x = sum_j s_j 16^-j (exact to ~2^-24)
    blh8 = din("blh8", [P, KRP * nkc], mybir.dt.float8e4)

    # output in [j, i] layout, bf16; host transposes while unsharding
    out = nc.dram_tensor("out", [nj, n], BF16, kind="ExternalOutput").ap()

    # exchange payload (bf16): the -B splits in (s, t, p) order
    nsp = 3 * nihc                  # -B split columns per partition
    npay = P * nsp
    bh_dram = nc.dram_tensor("bh_dram", [1, npay], BF16).ap()
    nhalves = 1 if single else 2
    bfull_dram = nc.dram_tensor("bfull_dram", [nhalves, npay], BF16).ap()
    groups = [[2 * p, 2 * p + 1] for p in range(max(1, num_devices // 2))]

    def mm512(out_ap, lhsT, rhs, start=True, stop=True):
        """matmul with the moving dim split into <=512-column chunks."""
        nfree = rhs.shape[-1]
        assert out_ap.shape[-1] == nfree
        for o in range(0, nfree, 512):
            e = min(o + 512, nfree)
            nc.tensor.matmul(
                out_ap[..., o:e], lhsT, rhs[..., o:e], start=start, stop=stop
            )

    with tile.TileContext(nc) as tc, ExitStack() as ctx:
        cpool = ctx.enter_context(tc.tile_pool(name="consts", bufs=1))

        def load(pool, ap_dram, shape, dt, name):
            t = pool.tile(shape, dt, tag=name)
            nc.sync.dma_start(out=t[:], in_=ap_dram)
            return t

        # input loads, critical-path first (xb halved so chunk-0 comparisons
        # can start on the first half)
        xb = cpool.tile([P, nih], F32, tag="xb")
        nc.sync.dma_start(out=xb[:, 0 : nih // 2], in_=xbf[:, 0 : nih // 2])
        pkf_s = load(cpool, pkf, [P, wf], F32, "pkf")
        nc.sync.dma_start(out=xb[:, nih // 2 : nih], in_=xbf[:, nih // 2 : nih])
        pkb_s = load(cpool, pkb, [P, wb], BF16, "pkb")
        l9 = load(cpool, l9full, [9, nj], BF16, "l9")
        r9 = [
            cpool.tile([9, nih], BF16, tag=f"r9_{h}", name=f"r9_{h}")
            for h in range(nhalves)
        ]
        nc.sync.dma_start(out=r9[0][3:9, :], in_=xr6)
        if nhalves == 2:
            nc.sync.dma_start(out=r9[1][3:9, :], in_=xr6o)
        rep9 = cpool.tile([9, NB], BF16, tag="rep9")
        nmcol = cpool.tile([P, n // P if single else n // 2 // P], F32,
                           tag="nmcol")

        xcol_s = pkf_s[:, of_xcol : of_xcol + nkc]
        xhc_s = pkf_s[:, of_xhc : of_xhc + nihc]
        lob_s = pkf_s[:, of_lob : of_lob + NB]
        hib_s = pkf_s[:, of_hib : of_hib + NB]
        i7f_s = pkf_s[0:7, of_i6 : of_i6 + 7]
        blh8_s = cpool.tile(
            [P, nkc // 2, 2, KRP], mybir.dt.float8e4, tag="blh8"
        )
        nc.sync.dma_start(
            out=blh8_s[:],
            in_=blh8.rearrange("p (a b c) -> p a b c", b=2, c=KRP),
        )
        xc2d_s = pkb_s[:, ob_xc2d : ob_xc2d + 2 * n_islt]
        i128_s = pkb_s[:, ob_i128 : ob_i128 + P]
        onesc_s = pkb_s[:, ob_onesc : ob_onesc + 1]

        # ---- PE warm-up: keep the p-state ramp going while inputs load,
        # and preload the ACT function table with a dummy Sign ----
        # SO-loop SBUF pools created BEFORE the prep pool so their tiles
        # never alias prep scratch (aliasing would chain the first exp
        # behind the last prep op through an SBUF reuse WAR)
        dpool = ctx.enter_context(tc.tile_pool(name="dd", bufs=6))
        outp = ctx.enter_context(tc.tile_pool(name="outp", bufs=5))
        wt = cpool.tile([P, 512], BF16, tag="wt")
        with (
            tc.tile_pool(name="warmp", bufs=1, space="PSUM") as wpp,
        ):
            nc.vector.memset(wt[:], 1.0)
            wsg = cpool.tile([1, 1], BF16, tag="wsg")
            nc.scalar.activation(out=wsg[:], in_=wt[0:1, 0:1], func=AF.Sign)
            wps = wpp.tile([P, 512], F32)
            for _ in range(10):
                nc.tensor.matmul(
                    wps[:], wt[:, 0:P], wt[:], start=True, stop=True
                )

        with tc.tile_pool(name="prep", bufs=1) as pp_s:
            # comparison pass across DVE/Pool (is_lt -> {0,1}) and ACT
            # (Sign -> {-1,0,1}); PE drains g tiles in program order. For
            # the is_lt set, sum_k |x_i-x_k| = x_i*(nD - 2*cntD) - SD +
            # 2*tD; for the ACT set it's x_i*sgnS - tS. Ties vanish either
            # way.
            nxcol = pp_s.tile([P, nkc], F32, tag="nxcol")
            nc.vector.tensor_scalar_mul(nxcol[:], xcol_s, -1.0)
            b3 = pp_s.tile([KR, nih], F32, tag="b3")
            b3s = pp_s.tile([KR, nih], F32, tag="b3s")
            npr_lt = n_islt // 2
            npr_sg = (nkc - n_islt) // 2
            with (
                tc.tile_pool(name="bp", bufs=1, space="PSUM") as bp,
                tc.tile_pool(name="gp", bufs=6) as gp,
            ):
                bpsum = bp.tile([KRP, nih], F32)
                bpsum2 = bp.tile([KRP, nih], F32)
                nlt_seen = nact_seen = 0
                for pi, (ptyp, (k0, k1)) in enumerate(pair_seq):
                    g = gp.tile([P, 2, nih], mybir.dt.float8e4, tag="g")
                    for sl, k in ((0, k0), (1, k1)):
                        if ptyp == "d":
                            spans = (
                                [(0, nih // 2), (nih // 2, nih)]
                                if pi == 0
                                else [(0, nih)]
                            )
                            for a0, a1 in spans:
                                nc.vector.tensor_scalar(
                                    out=g[:, sl, a0:a1],
                                    in0=xb[:, a0:a1],
                                    scalar1=xcol_s[:, k : k + 1],
                                    scalar2=None,
                                    op0=ALU.is_lt,
                                )
                        else:
                            nc.scalar.activation(
                                out=g[:, sl, :],
                                in_=xb[:],
                                func=AF.Sign,
                                bias=nxcol[0:P, k : k + 1],
                            )
                    acc = bpsum if ptyp == "d" else bpsum2
                    if ptyp == "d":
                        nlt_seen += 1
                        first, last = nlt_seen == 1, nlt_seen == npr_lt
                    else:
                        nact_seen += 1
                        first, last = nact_seen == 1, nact_seen == npr_sg
                    # fp8 DoubleRow: both chunks of the pair reduced in one
                    # matmul stream at 0.5 cyc/row
                    lhsT = blh8_s[:, pi, :, :]
                    for o in range(0, nih, 512):
                        nc.tensor.matmul(
                            acc[:, o : o + 512],
                            lhsT,
                            g[:, :, o : o + 512],
                            perf_mode=mybir.MatmulPerfMode.DoubleRow,
                            start=first,
                            stop=last,
                        )
                # drain both accumulators, split across DVE and ACT so the
                # copies overlap (compute APs must start at partition 0)
                hw2 = nih // 2
                nc.vector.tensor_copy(
                    out=b3[:, 0:hw2], in_=bpsum[0:KR, 0:hw2]
                )
                nc.scalar.activation(
                    out=b3[:, hw2:nih], in_=bpsum[0:KR, hw2:nih], func=AF.Copy
                )
                nc.vector.tensor_copy(
                    out=b3s[:, 0:hw2], in_=bpsum2[0:KR, 0:hw2]
                )
                nc.scalar.activation(
                    out=b3s[:, hw2:nih], in_=bpsum2[0:KR, hw2:nih],
                    func=AF.Copy,
                )

            # rows -> columns via tiny PE transposes (exact data movement)
            bc_all = pp_s.tile([P, nihc, 2 * KR], F32, tag="bc_all")
            with tc.tile_pool(name="tp", bufs=1, space="PSUM") as tp:
                bc_ps = tp.tile([P, nihc, 2 * KR], F32)
                for ch in range(nihc):
                    nc.tensor.transpose(
                        bc_ps[:, ch, 0:KR],
                        b3[:, ch * P : (ch + 1) * P],
                        i7f_s,
                    )
                    nc.tensor.transpose(
                        bc_ps[:, ch, KR : 2 * KR],
                        b3s[:, ch * P : (ch + 1) * P],
                        i7f_s,
                    )
                nc.vector.tensor_copy(out=bc_all[:], in_=bc_ps[:])
            cntc = bc_all[:, :, 0]
            sgnc = bc_all[:, :, KR]
            # Horner-recombine the scaled split rows: t = sum_j 16^-j t_j
            tdc = pp_s.tile([P, nihc], F32, tag="tdc")
            tsc = pp_s.tile([P, nihc], F32, tag="tsc")
            for dst, base in ((tdc, 0), (tsc, KR)):
                nc.vector.tensor_copy(out=dst[:], in_=bc_all[:, :, base + 6])
                for j in range(5, 0, -1):
                    nc.vector.scalar_tensor_tensor(
                        out=dst[:],
                        in0=dst[:],
                        scalar=1.0 / 16.0,
                        in1=bc_all[:, :, base + j],
                        op0=ALU.mult,
                        op1=ALU.add,
                    )

            # +SD (sum of x over the is_lt-chunk k's)
            spos = pp_s.tile([1, 1], F32, tag="spos")
            with tc.tile_pool(name="sp", bufs=1, space="PSUM") as sp:
                sxp = sp.tile([1, 2 * n_islt], F32)
                nc.tensor.matmul(sxp[:], onesc_s, xc2d_s, start=True, stop=True)
                nc.vector.tensor_reduce(
                    out=spos[:], in_=sxp[:], axis=mybir.AxisListType.X, op=ALU.add
                )
            sposc = pp_s.tile([P, 1], F32, tag="sposc")
            nc.gpsimd.partition_broadcast(sposc[:], spos[0:1, 0:1])

            # -B = -x*(nD - 2*cntD + sgnS) + SD - 2*(tDh+tDl) + (tSh+tSl)
            nD = float(n_islt * P)
            r1 = pp_s.tile([P, nihc], F32, tag="r1")
            nc.vector.tensor_scalar(
                out=r1[:],
                in0=cntc,
                scalar1=-2.0,
                scalar2=nD,
                op0=ALU.mult,
                op1=ALU.add,
            )
            r1b = pp_s.tile([P, nihc], F32, tag="r1b")
            nc.vector.tensor_tensor(out=r1b[:], in0=r1[:], in1=sgnc, op=ALU.add)
            r2n = pp_s.tile([P, nihc], F32, tag="r2n")
            nc.vector.scalar_tensor_tensor(
                out=r2n[:], in0=xhc_s, scalar=-1.0, in1=r1b[:],
                op0=ALU.mult, op1=ALU.mult,
            )
            u1 = pp_s.tile([P, nihc], F32, tag="u1")
            nc.vector.scalar_tensor_tensor(
                out=u1[:], in0=tdc[:], scalar=-2.0, in1=r2n[:],
                op0=ALU.mult, op1=ALU.add,
            )
            u2 = pp_s.tile([P, nihc], F32, tag="u2")
            nc.vector.tensor_tensor(out=u2[:], in0=u1[:], in1=tsc[:], op=ALU.add)
            nbhalfc = pp_s.tile([P, nihc], F32, tag="nbhalfc")
            nc.vector.tensor_scalar(
                out=nbhalfc[:],
                in0=u2[:],
                scalar1=sposc[:, 0:1],
                scalar2=None,
                op0=ALU.add,
            )

            # -B bf16 splits, chunk-major [P, t, s] so each chunk's three
            # split columns sit adjacent for the PE row-transposes below
            nbsh = pp_s.tile([P, nihc, 3], BF16, tag="nbsh")
            sp0 = nbsh[:, :, 0]
            sp1 = nbsh[:, :, 1]
            sp2 = nbsh[:, :, 2]
            cs_t1 = pp_s.tile([P, nihc], F32, tag="cs_t1")
            cs_t2 = pp_s.tile([P, nihc], F32, tag="cs_t2")
            nc.vector.tensor_copy(out=sp0, in_=nbhalfc[:])
            nc.vector.tensor_tensor(out=cs_t1[:], in0=nbhalfc[:], in1=sp0,
                                    op=ALU.subtract)
            nc.vector.tensor_copy(out=sp1, in_=cs_t1[:])
            nc.vector.tensor_tensor(out=cs_t2[:], in0=cs_t1[:], in1=sp1,
                                    op=ALU.subtract)
            nc.vector.tensor_copy(out=sp2, in_=cs_t2[:])

            # ---- -B rows straight into r9[0] via tiny PE transposes (PE
            # and ACT are idle here; skips the slow element-scatter DMA),
            # then one contiguous DMA publishes them as the payload ----
            with tc.tile_pool(name="btp", bufs=1, space="PSUM") as btp:
                btr = btp.tile([3, nih], BF16)
                for ch in range(nihc):
                    nc.tensor.transpose(
                        btr[:, ch * P : (ch + 1) * P], nbsh[:, ch, :], i128_s
                    )
                nc.scalar.activation(out=r9[0][0:3, :], in_=btr[:], func=AF.Copy)
            nc.sync.dma_start(
                out=bh_dram[0, 0 : P * nsp].rearrange("(s i) -> s i", s=3),
                in_=r9[0][0:3, :],
            )
            if use_collective:
                nc.gpsimd.collective_compute(
                    "AllGather",
                    ALU.bypass,
                    replica_groups=groups,
                    ins=[bh_dram],
                    outs=[bfull_dram],
                )
            else:
                for hh in range(nhalves):
                    nc.sync.dma_start(out=bfull_dram[hh : hh + 1, :], in_=bh_dram)

            if nhalves == 2:
                # AllGather slots are by group position, so slot 1 is this
                # very core on odd ranks; the payload slabs are read here
                # and the position-free recovery runs AFTER the M' chain so
                # the collective never blocks the own-half exp stream.
                s01 = pp_s.tile([3, 2, nih], BF16, tag="s01")
                nc.sync.dma_start(
                    out=s01[:],
                    in_=bfull_dram[0:2, 0 : P * nsp].rearrange(
                        "h (s i) -> s h i", s=3
                    ),
                )

            # ---- own-half rank-bucket partial sums (pre-exchange) ----
            xballh_s = load(pp_s, xballh, [P, nihc, 5], BF16, "xballh")
            nc.vector.tensor_copy(out=xballh_s[:, :, 2], in_=nbsh[:, :, 0])
            nc.vector.tensor_copy(out=xballh_s[:, :, 3], in_=nbsh[:, :, 1])
            reps_own = pp_s.tile([NB, 5], F32, tag="reps_own")
            with (
                tc.tile_pool(name="repp0", bufs=1, space="PSUM") as repp0,
                tc.tile_pool(name="mkp0", bufs=8) as mkp0,
            ):
                repso_p = repp0.tile([NB, 5], F32)
                nrch = max(1, (3 * nihc) // 4)  # a subset stays a valid
                for ch in range(nrch):          # max-underestimate (convexity)
                    m1 = mkp0.tile([P, NB], BF16, tag="m1")
                    nc.vector.tensor_scalar(
                        out=m1[:],
                        in0=lob_s,
                        scalar1=r1b[:, ch : ch + 1],
                        scalar2=None,
                        op0=ALU.is_le,
                    )
                    msk = mkp0.tile([P, NB], BF16, tag="msk")
                    nc.vector.scalar_tensor_tensor(
                        out=msk[:],
                        in0=hib_s,
                        scalar=r1b[:, ch : ch + 1],
                        in1=m1[:],
                        op0=ALU.is_gt,
                        op1=ALU.mult,
                    )
                    nc.tensor.matmul(
                        repso_p[:],
                        msk[:],
                        xballh_s[:, ch, :],
                        start=(ch == 0),
                        stop=(ch == nrch - 1),
                    )
                nc.vector.tensor_copy(out=reps_own[:], in_=repso_p[:])
            # ---- rank-bucket representatives -> rep9 stack ----
            # OWN-half reps only: the top ranks of every column are never
            # all in the partner half (P ~ 2^-30), so the own-half bucket
            # max underestimates each column max by only a few more units
            # than the full-data version -- still far inside the exp(88)
            # bf16 budget, and softmax shift-invariance keeps the result
            # exact. This takes the whole M' pipeline off the exchange
            # critical path.
            # reps rows: [sum xh, sum xl, sum -Bh, sum -Bm, count]
            reps = reps_own
            cnt1 = pp_s.tile([NB, 1], F32, tag="cnt1")
            nc.vector.tensor_scalar_max(cnt1[:], reps[:, 4:5], 1.0)
            rc = pp_s.tile([NB, 1], F32, tag="rc")
            nc.vector.reciprocal(rc[:], cnt1[:])
            repx = pp_s.tile([NB, 1], F32, tag="repx")
            nc.vector.tensor_tensor(
                out=repx[:], in0=reps[:, 0:1], in1=reps[:, 1:2], op=ALU.add
            )
            nc.vector.tensor_tensor(
                out=repx[:], in0=repx[:], in1=rc[:], op=ALU.mult
            )
            repb = pp_s.tile([NB, 1], F32, tag="repb")  # mean of -B
            nc.vector.tensor_tensor(
                out=repb[:], in0=reps[:, 2:3], in1=reps[:, 3:4], op=ALU.add
            )
            nc.vector.tensor_tensor(
                out=repb[:], in0=repb[:], in1=rc[:], op=ALU.mult
            )
            # empty bucket -> push its line to -inf
            iz = pp_s.tile([NB, 1], F32, tag="iz")
            nc.vector.tensor_scalar(
                out=iz[:], in0=reps[:, 4:5], scalar1=0.5, scalar2=None,
                op0=ALU.is_le,
            )
            nc.vector.scalar_tensor_tensor(
                out=repb[:], in0=iz[:], scalar=-1e30, in1=repb[:],
                op0=ALU.mult, op1=ALU.add,
            )
            # rep9 columns pre-transpose: [nBh2,nBm2,0,xh2,xh2,xl2,xl2,0,0]
            rs9 = pp_s.tile([NB, 9], BF16, tag="rs9")
            rtmp = pp_s.tile([NB, 1], F32, tag="rep_rt")
            nc.vector.tensor_copy(out=rs9[:, 0:1], in_=repb[:])
            nc.vector.tensor_tensor(
                out=rtmp[:], in0=repb[:], in1=rs9[:, 0:1], op=ALU.subtract
            )
            nc.vector.tensor_copy(out=rs9[:, 1:2], in_=rtmp[:])
            nc.vector.memset(rs9[:, 2:3], 0.0)
            nc.vector.tensor_copy(out=rs9[:, 3:4], in_=repx[:])
            nc.vector.tensor_copy(out=rs9[:, 4:5], in_=rs9[:, 3:4])
            nc.vector.tensor_tensor(
                out=rtmp[:], in0=repx[:], in1=rs9[:, 3:4], op=ALU.subtract
            )
            nc.vector.tensor_copy(out=rs9[:, 5:6], in_=rtmp[:])
            nc.vector.tensor_copy(out=rs9[:, 6:7], in_=rs9[:, 5:6])
            nc.vector.memset(rs9[:, 7:9], 0.0)
            with tc.tile_pool(name="repp", bufs=1, space="PSUM") as repp:
                p9r = repp.tile([9, NB], F32)
                nc.tensor.matmul(
                    p9r[:], rs9[:], i128_s[0:NB, 0:NB], start=True, stop=True
                )
                nc.vector.tensor_copy(out=rep9[:], in_=p9r[:])

            # ---- M' for every j-chunk upfront (needs only rep9 + l9, so
            # this overlaps the collective): z at the 128 bucket reps per
            # chunk, DVE max-reduces, negate. The placeholder pool pins
            # zrep to the upper PSUM banks so the SO loop's first z tile
            # (lower banks) doesn't wait for the M' reduces. ----
            with tc.tile_pool(name="mrp", bufs=1, space="PSUM") as mrp:
                zrep = mrp.tile([P, njc, NB], F32)
                for jc in range(njc):
                    nc.tensor.matmul(
                        zrep[:, jc, :],
                        l9[:, jc * P : (jc + 1) * P],
                        rep9[:],
                        start=True,
                        stop=True,
                    )
                mcol = pp_s.tile([P, njc], F32, tag="mcol")
                nsp0 = min(4, njc)
                nc.vector.tensor_reduce(
                    out=mcol[:, 0:nsp0], in_=zrep[:, 0:nsp0, :],
                    axis=mybir.AxisListType.X, op=ALU.max,
                )
                nc.vector.tensor_scalar_mul(
                    nmcol[:, 0:nsp0], mcol[:, 0:nsp0], -1.0
                )
                if nhalves == 2:
                    # partner -B rows = (slab0 + slab1) - own: exact, since
                    # within-row magnitudes are homogeneous bf16 values so
                    # the f32 sums round-trip exactly. Ordered between the
                    # M' reduces: the first chunks' bias is urgent, the
                    # rest isn't, and q1 fills need these rows soon.
                    ssum2 = pp_s.tile([3, nih], F32, tag="ssum2")
                    nc.vector.tensor_tensor(
                        out=ssum2[:], in0=s01[:, 0, :], in1=s01[:, 1, :],
                        op=ALU.add,
                    )
                    nc.vector.tensor_tensor(
                        out=r9[1][0:3, :], in0=ssum2[:], in1=r9[0][0:3, :],
                        op=ALU.subtract,
                    )
                if njc > nsp0:
                    nc.vector.tensor_reduce(
                        out=mcol[:, nsp0:njc], in_=zrep[:, nsp0:njc, :],
                        axis=mybir.AxisListType.X, op=ALU.max,
                    )
                    nc.vector.tensor_scalar_mul(
                        nmcol[:, nsp0:njc], mcol[:, nsp0:njc], -1.0
                    )

        # ---------------- Phase SO: merged softmax+output per j-chunk -------
        # software-pipelined half-streams: the own-half (q=0) exp of chunk c
        # is issued before the partner-half (q=1) exp of chunk c-1, so the
        # ACT stream starts as soon as r9[0] lands -- before the collective
        # delivers r9[1].
        spool = ctx.enter_context(tc.tile_pool(name="sz", bufs=2, space="PSUM"))
        nhi = n // ih  # i-halves per chunk

        def z_half(zp, lhs, q):
            h, qq = divmod(q * ih, nih)
            o = 0
            while o < ih:
                hh, qo = h, qq + o
                if qo >= nih:
                    hh, qo = h + 1, qo - nih
                e = min(qo + 512, nih) - qo
                nc.tensor.matmul(
                    zp[:, o : o + e],
                    lhs,
                    r9[hh][:, qo : qo + e],
                    start=True,
                    stop=True,
                )
                o += e

        def finalize(st):
            ot, dq = st["ot"], st["dq"]
            dsum = dpool.tile([P, 1], F32, tag="dsum")
            nc.vector.tensor_tensor(
                out=dsum[:], in0=dq[:, 0:1], in1=dq[:, 1:2], op=ALU.add
            )
            rcp = dpool.tile([P, 1], F32, tag="rcp")
            nc.vector.reciprocal(rcp[:], dsum[:])
            npc = 4 if st["jc"] == njc - 1 else 2
            for hh in range(npc):
                sl = slice(hh * (n // npc), (hh + 1) * (n // npc))
                nc.vector.tensor_scalar(
                    out=ot[:, sl],
                    in0=ot[:, sl],
                    scalar1=rcp[:, 0:1],
                    scalar2=None,
                    op0=ALU.mult,
                )
                nc.sync.dma_start(
                    out=out.rearrange("(jc p) i -> p jc i", p=P)[
                        :, st["jc"], sl
                    ],
                    in_=ot[:, sl],
                )

        skew = spool.tile([P, ih], F32, tag="sz", name="skew")  # rotation
        # skew: the first real z tile must not alias the M' zrep banks
        DEPTH = 2  # own-half stream runs this many chunks ahead of the
        pending = []  # partner-half stream (collective latency headroom)
        for jc in range(njc + DEPTH):
            if jc < njc:
                lhs = l9[:, jc * P : (jc + 1) * P]
                st = {
                    "jc": jc,
                    "ot": outp.tile([P, n], BF16, tag="ot", name="ot"),
                    "dq": dpool.tile([P, 2], F32, tag="dq", name="dq"),
                    "lhs": lhs,
                }
                zp = spool.tile([P, ih], F32, tag="sz")
                z_half(zp, lhs, 0)
                nc.scalar.activation(
                    out=st["ot"][:, 0:ih],
                    in_=zp[:],
                    func=AF.Exp,
                    bias=nmcol[0:P, jc : jc + 1],
                    scale=1.0,
                    accum_out=st["dq"][:, 0:1],
                )
                pending.append(st)
            if len(pending) > DEPTH or jc >= njc:
                prev = pending.pop(0)
                zp1 = spool.tile([P, ih], F32, tag="sz")
                z_half(zp1, prev["lhs"], 1)
                nc.scalar.activation(
                    out=prev["ot"][:, ih : 2 * ih],
                    in_=zp1[:],
                    func=AF.Exp,
                    bias=nmcol[0:P, prev["jc"] : prev["jc"] + 1],
                    scale=1.0,
                    accum_out=prev["dq"][:, 1:2],
                )
                finalize(prev)

    nc.compile()
    return nc


# ---------------------------------------------------------------------------


def make_in_maps(scores, n, mode="pair"):
    """Per-core input dicts. Core c -> batch c//2, halves h = c%2."""
    single = mode == "single"
    nj = n if single else n // 2
    nih = n if single else n // 2
    nkc = n // P
    nihc = nih // P
    ncores = 1 if single else N_CORES

    cfull = (2 * np.arange(n) + 1 - n).astype(np.float32)
    ch_f, cl_f = _split2(cfull)

    islt_ks = _islt_ks(nkc)
    n_islt = len(islt_ks)

    in_maps = []
    for c in range(ncores):
        b = 0 if single else c // 2
        h = 0 if single else c % 2
        x = np.asarray(scores[b], dtype=np.float32)
        xh_, xm_, xl_ = _split3(x)
        xch, xcl = _split2(x)
        xcol = np.ascontiguousarray(x.reshape(nkc, P).T).astype(np.float32)
        xchc = np.ascontiguousarray(xch.reshape(nkc, P).T)
        xclc = np.ascontiguousarray(xcl.reshape(nkc, P).T)
        # fp8 pair-ordered stationaries: per chunk cols [1, s0..s5]
        KR = 7
        sign_ks = [k for k in range(nkc) if k not in islt_ks]
        di = [(islt_ks[i], islt_ks[i + 1]) for i in range(0, n_islt, 2)]
        ai = [(sign_ks[i], sign_ks[i + 1]) for i in range(0, len(sign_ks), 2)]
        pat = ["d", "a", "d", "d", "a", "d", "a", "d"]
        pair_seq = []
        while di or ai:
            for c in pat:
                if c == "d" and di:
                    pair_seq.append(di.pop(0))
                elif c == "a" and ai:
                    pair_seq.append(ai.pop(0))
        KRP = 16
        s8 = _split8(xcol)  # list of 6 [P, nkc] fp8 arrays
        blh8 = np.zeros((P, KRP * nkc), dtype=ml_dtypes.float8_e4m3fn)
        for pi, (k0, k1) in enumerate(pair_seq):
            for sl, k in ((0, k0), (1, k1)):
                base = KRP * (2 * pi + sl)
                blh8[:, base] = 1.0
                for j in range(6):
                    blh8[:, base + 1 + j] = s8[j][:, k]
        xc2d = np.concatenate([xchc[:, islt_ks], xclc[:, islt_ks]], axis=1)
        assert xc2d.shape[1] == 2 * n_islt
        sl = slice(h * nih, h * nih + nih)
        sj = slice(h * nj, h * nj + nj)
        so = slice((1 - h) * nih, (1 - h) * nih + nih) if not single else sl

        def xr6_of(s):
            return np.stack(
                [xh_[s], xh_[s], xm_[s], xm_[s], xl_[s], xl_[s]], axis=0
            )

        ones_j = np.ones((3, nj), dtype=ml_dtypes.bfloat16)
        l9full = np.concatenate(
            [
                ones_j,
                ch_f[None, sj], cl_f[None, sj],
                ch_f[None, sj], cl_f[None, sj],
                ch_f[None, sj], cl_f[None, sj],
            ],
            axis=0,
        )
        NB = 64
        lo_row = (-n + np.arange(NB) * (2 * n // NB)).astype(np.float32)
        lob = np.tile(lo_row[None, :], (P, 1))
        hib = lob + float(2 * n // NB)
        xballh = np.zeros((P, nihc, 5), dtype=ml_dtypes.bfloat16)
        xballh[:, :, 0] = xchc[:, h * nihc : (h + 1) * nihc]
        xballh[:, :, 1] = xclc[:, h * nihc : (h + 1) * nihc]
        xballh[:, :, 4] = 1.0

        wb = 2 * n_islt + P + 1
        pkb = np.zeros((P, wb), dtype=ml_dtypes.bfloat16)
        o = 0
        pkb[:, o : o + 2 * n_islt] = xc2d
        o += 2 * n_islt
        pkb[:, o : o + P] = np.eye(P, dtype=ml_dtypes.bfloat16)
        o += P
        pkb[:, o] = 1.0  # onesc
        o += 1
        assert o == wb

        wf = nkc + nihc + NB + NB + 7
        pkf = np.zeros((P, wf), dtype=np.float32)
        o = 0
        pkf[:, o : o + nkc] = xcol
        o += nkc
        pkf[:, o : o + nihc] = np.ascontiguousarray(x[sl].reshape(-1, P).T)
        o += nihc
        pkf[:, o : o + NB] = lob
        o += NB
        pkf[:, o : o + NB] = hib
        o += NB
        pkf[0:7, o : o + 7] = np.eye(7, dtype=np.float32)
        o += 7
        assert o == wf

        in_maps.append(
            {
                "xbf": np.tile(x[sl][None, :], (P, 1)),
                "l9full": l9full,
                "pkf": pkf,
                "pkb": pkb,
                "xr6": xr6_of(sl),
                "xr6o": xr6_of(so),
                "xballh": xballh,
                "blh8": blh8,
            }
        )
    return in_maps


_NC_CACHE = {}


def _get_nc(n):
    if n not in _NC_CACHE:
        _NC_CACHE[n] = build_nc(n=n, mode="pair", num_devices=N_CORES)
    return _NC_CACHE[n]


def kernel(scores):
    scores = np.asarray(scores, dtype=np.float32)
    b, n = scores.shape
    nj = n // 2
    nih = n // 2
    nc = _get_nc(n)
    in_maps = make_in_maps(scores, n, mode="pair")
    res = run_bass_kernel_spmd(nc, in_maps, list(range(N_CORES)))
    out = np.empty((b, n, n), dtype=np.float32)
    for c in range(N_CORES):
        bb, h = c // 2, c % 2
        odev = np.asarray(res.results[c]["out"], dtype=np.float32)  # [nj, n]
        # odev columns: [own half (i in h-half) | partner half]
        out[bb, h * nih : (h + 1) * nih, h * nj : (h + 1) * nj] = odev[
            :, 0:nih
        ].T
        out[bb, (1 - h) * nih : (2 - h) * nih, h * nj : (h + 1) * nj] = odev[
            :, nih : 2 * nih
        ].T
    return out
